# revision 1
# baseline (speedup 1.0000x reference)
"""Multi-head self-attention (b=2, n=2048, d_model=1024, 8 heads x 64) on 8 TRN2 cores.

Sharding: token-parallel (512 tokens/core, batch-major), K/V exchanged via two
4-rank AllGathers (replica groups = batch element). Everything is computed in
layouts that avoid transposing the attention matrix:

  xT    [1024, 512]  (PE-transposed from x shard)
  QT/KT [512(inner), tokens] = W.T @ xT   (matmul lhsT=W chunk, rhs=xT chunk)
  V_aug [tokens, 8*(64+1)]   = xT.T @ Wv  (+ ones column per head)
  scoresT[keys,q]  = matmul(lhsT=KT[64,128], rhs=QT[64,512])
  expT   = ACT exp(0.125*scoresT)  PSUM->SBUF
  outT[65,q]      += matmul(lhsT=V_aug[128,65], rhs=expT[128,512])  (row 64 = sumexp)
  normalize via DVE reciprocal + K=1 broadcast matmul
  y[tok,1024]      = matmul(lhsT=aoutT[128,128], rhs=Wo[128,512]) + ones x bo
"""

import numpy as np

import concourse.bass as bass
import concourse.mybir as mybir
import concourse.tile as tile
from concourse import bacc
from concourse.bass_utils import run_bass_kernel_spmd
from concourse.masks import make_identity

F32 = mybir.dt.float32
FR = mybir.dt.float32r
BF = mybir.dt.bfloat16

B, S, D = 2, 2048, 1024
H, DH = 8, 64
INNER = H * DH            # 512
N_CORES = 8
GROUP = 4                 # cores per batch element
TOK = (B * S) // N_CORES  # 512 tokens per core
NKB = S // 128            # 16 key blocks per batch context
SCALE = DH ** -0.5        # 0.125

REPLICA_GROUPS = [[0, 1, 2, 3], [4, 5, 6, 7]]

_CACHE = {}
NO_COLLECTIVE = False   # timing A/B switch (wrong math, same local work)
SKIP_ATTN = False       # timing bisect: skip attention phase
SKIP_PROJ = False       # timing bisect: skip startup/projection phase
EXP_AS_COPY = False     # timing probe: stage scores via DVE copy instead of ACT exp


def _build_kernel(no_collective=False, reps=1):
    nc = bacc.Bacc("TRN2", target_bir_lowering=False, debug=False,
                   num_devices=N_CORES)

    x_d = nc.dram_tensor("x_shard", [TOK, D], F32, kind="ExternalInput")
    wq_d = nc.dram_tensor("Wq", [D, INNER], F32, kind="ExternalInput")
    wkv_d = nc.dram_tensor("Wkv", [D, 2 * INNER], F32, kind="ExternalInput")
    wo_d = nc.dram_tensor("Wo", [INNER, D], F32, kind="ExternalInput")
    bo_d = nc.dram_tensor("bo", [D], F32, kind="ExternalInput")
    y_d = nc.dram_tensor("y_shard", [TOK, D], F32, kind="ExternalOutput")

    # fused collective bounce buffer: rows 0-511 = KT shard (512 of 520
    # cols used), rows 512-1023 = V_aug shard
    W_AG = H * 65  # 520
    agkv_in = nc.dram_tensor("agkv_in", [INNER + TOK, W_AG], BF,
                             kind="Internal")
    agkv_out = nc.dram_tensor("agkv_out", [GROUP * (INNER + TOK), W_AG], BF,
                              kind="Internal")

    with tile.TileContext(nc) as tc:
        for _ in range(reps):
            _trace_body(nc, tc, x_d, wq_d, wkv_d, wo_d, bo_d, y_d,
                        agkv_in, agkv_out,
                        no_collective=no_collective)

    nc.compile()
    return nc


def _trace_body(nc, tc, x_d, wq_d, wkv_d, wo_d, bo_d, y_d,
                agkv_in, agkv_out, no_collective=False):
    Exp = mybir.ActivationFunctionType.Exp

    def fr(ap):
        # float32r: TF32-like PE mode, 4x matmul throughput vs float32
        return ap.bitcast(mybir.dt.float32r)

    def all_gather(in_t, out_t, nrows):
        if no_collective:
            for r in range(GROUP):
                nc.sync.dma_start(
                    out_t.ap()[r * nrows:(r + 1) * nrows, :], in_t.ap())
        else:
            nc.gpsimd.collective_compute(
                "AllGather", mybir.AluOpType.bypass,
                replica_groups=REPLICA_GROUPS,
                ins=[in_t.ap()], outs=[out_t.ap()])

    # score-psum wave layout: kb blocks per wave (2 banks double-buffered)
    WAVES = [(k, 2) for k in range(0, NKB, 2)]
    WAVE_NKB = 2

    with (
        tc.tile_pool(name="const", bufs=1) as constp,
        tc.tile_pool(name="wo", bufs=1) as wop,
        tc.tile_pool(name="qt", bufs=1) as qtp,
        tc.tile_pool(name="stage", bufs=3) as stagep,
        tc.tile_pool(name="expt", bufs=3) as expp,
        tc.tile_pool(name="ao", bufs=1) as aop,
        tc.tile_pool(name="ys", bufs=2) as ysp,
        tc.tile_pool(name="small", bufs=2) as smallp,
    ):
        # ---- constants ----
        ident = constp.tile([128, 128], F32, tag="ident")
        make_identity(nc, ident[:])
        ones_f = constp.tile([1, 128], F32, tag="onesf")
        nc.gpsimd.memset(ones_f[:], 1.0)
        ones = constp.tile([1, 128], BF, tag="ones")
        nc.vector.tensor_copy(ones[:], ones_f[:])
        bo_sb = constp.tile([1, D], BF, tag="bo")
        nc.gpsimd.dma_start(
            bo_sb[:], bo_d.ap().rearrange("(a n) -> a n", a=1))

        # ---- persistent activations ----
        qt_sb = qtp.tile([128, 4, TOK], BF, tag="qt")          # QT [inner, tok]
        aout_sb = aop.tile([128, 4, TOK], BF, tag="aout")      # attnT out [inner, tok]

        if SKIP_PROJ:
            nc.vector.memset(qt_sb[:], 0.01)
        if SKIP_ATTN:
            nc.vector.memset(aout_sb[:], 0.01)
        if not SKIP_PROJ:
          with (
            tc.tile_pool(name="xp", bufs=2) as xp,
            tc.tile_pool(name="xtp", bufs=1) as xtp,
            tc.tile_pool(name="wq", bufs=1) as wqp,
            tc.tile_pool(name="wkv", bufs=1) as wkvp,
            tc.tile_pool(name="pworka", bufs=2, space="PSUM") as pworka,
          ):
            # ---- load x first (transposes gate everything), then Wkv ----
            xt_sb = xtp.tile([128, 8, TOK], BF, tag="xt")
            x_tiles = []
            for a in range(4):
                x_t = xp.tile([128, D], F32, tag="x")
                nc.sync.dma_start(x_t[:], x_d.ap()[a * 128:(a + 1) * 128, :])
                x_tiles.append(x_t)
            wkvk_sb = wkvp.tile([128, 8, INNER], BF, tag="wkvk")
            wkvv_sb = wkvp.tile([128, 8, INNER], BF, tag="wkvv")
            nc.gpsimd.dma_start(
                wkvk_sb[:],
                wkv_d.ap()[:, 0:INNER].rearrange("(c p) n -> p c n", p=128))

            # ---- transpose x shard: xT [1024, 512] ----
            for a in range(4):
                x_t = x_tiles[a]
                for c in range(8):
                    pt = pworka.tile([128, 128], F32, tag="work")
                    nc.tensor.transpose(pt[:], x_t[:, c * 128:(c + 1) * 128],
                                        ident[:])
                    nc.vector.tensor_copy(
                        xt_sb[:, c, a * 128:(a + 1) * 128], pt[:])

            # ---- K projection -> agk_in, AllGather ----
            for m in range(4):
                ps = pworka.tile([128, TOK], F32, tag="work")
                for c in range(8):
                    nc.tensor.matmul(ps[:],
                                     lhsT=wkvk_sb[:, c, m * 128:(m + 1) * 128],
                                     rhs=xt_sb[:, c, :],
                                     start=(c == 0), stop=(c == 7))
                st = stagep.tile([128, TOK], BF, tag="ktstage")
                nc.vector.tensor_copy(st[:], ps[:])
                nc.sync.dma_start(
                    agkv_in.ap()[m * 128:(m + 1) * 128, 0:TOK], st[:])

            # ---- V projection (+ones col) -> agv_in, AllGather ----
            nc.gpsimd.dma_start(
                wkvv_sb[:],
                wkv_d.ap()[:, INNER:2 * INNER]
                .rearrange("(c p) n -> p c n", p=128))
            for a in range(4):
                ps = pworka.tile([128, INNER], F32, tag="work")
                for c in range(8):
                    nc.tensor.matmul(ps[:],
                                     lhsT=xt_sb[:, c, a * 128:(a + 1) * 128],
                                     rhs=wkvv_sb[:, c, :],
                                     start=(c == 0), stop=(c == 7))
                vst = stagep.tile([128, H, 65], BF, tag="vstage")
                nc.vector.tensor_copy(
                    vst[:, :, 0:64], ps[:].rearrange("p (h e) -> p h e", e=64))
                nc.vector.memset(vst[:, :, 64:65], 1.0)
                nc.sync.dma_start(
                    agkv_in.ap()[INNER + a * 128:INNER + (a + 1) * 128, :]
                    .rearrange("p (h e) -> p h e", e=65),
                    vst[:])
            all_gather(agkv_in, agkv_out, INNER + TOK)

            # ---- Q projection ----
            wq_sb = wqp.tile([128, 8, INNER], BF, tag="wq")
            nc.gpsimd.dma_start(
                wq_sb[:], wq_d.ap().rearrange("(c p) n -> p c n", p=128))
            for m in range(4):
                ps = pworka.tile([128, TOK], F32, tag="work")
                for c in range(8):
                    nc.tensor.matmul(ps[:],
                                     lhsT=wq_sb[:, c, m * 128:(m + 1) * 128],
                                     rhs=xt_sb[:, c, :],
                                     start=(c == 0), stop=(c == 7))
                nc.vector.tensor_copy(qt_sb[:, m, :], ps[:])

        # ---- load gathered K/V per rank (pool opened after phase-A freed) ----
        kvp_cm = tc.tile_pool(name="kv", bufs=1)
        kvp = kvp_cm.__enter__()
        kt_all = kvp.tile([128, 4, GROUP, TOK], BF, tag="kt")  # KT [inner, keys]
        vaug_all = kvp.tile([128, NKB, H, 65], BF, tag="vaug")
        RB = INNER + TOK  # 1024 rows per rank in agkv_out
        for r in range(GROUP):
            nc.sync.dma_start(
                kt_all[:, :, r, :],
                agkv_out.ap()[r * RB:r * RB + INNER, 0:TOK]
                .rearrange("(m p) t -> p m t", p=128))
            nc.sync.dma_start(
                vaug_all[:, 4 * r:4 * (r + 1), :, :],
                agkv_out.ap()[r * RB + INNER:(r + 1) * RB, :]
                .rearrange("(kb p) (h e) -> p kb h e", p=128, e=65))

        # ---- attention per head ----
        wo_sb = wop.tile([128, 4, D], BF, tag="wo")
        nc.gpsimd.dma_start(
            wo_sb[:], wo_d.ap().rearrange("(c p) n -> p c n", p=128))
        pscorep_cm = tc.tile_pool(name="pscore", bufs=2, space="PSUM")
        pscorep = pscorep_cm.__enter__()
        pavp_cm = tc.tile_pool(name="pav", bufs=2, space="PSUM")
        pavp = pavp_cm.__enter__()
        pbp_cm = tc.tile_pool(name="pb", bufs=1, space="PSUM")
        pbp = pbp_cm.__enter__()
        # Software-pipelined: wave w+1's score matmuls are enqueued on PE
        # BEFORE wave w's AV matmuls, so PE never stalls behind ACT (strict
        # per-engine FIFO queues).
        def emit_scores(h, kb0, nkb):
            po, m = (h % 2) * 64, h // 2
            pscore = pscorep.tile([128, WAVE_NKB * TOK], F32, tag="s")
            for i in range(nkb):
                kb = kb0 + i
                nc.tensor.matmul(
                    pscore[:, i * TOK:(i + 1) * TOK],
                    lhsT=kt_all[po:po + 64, m, kb // 4,
                                (kb % 4) * 128:(kb % 4) * 128 + 128],
                    rhs=qt_sb[po:po + 64, m, :],
                    start=True, stop=True)
            return pscore

        def emit_exp(pscore, nkb):
            expt = expp.tile([128, WAVE_NKB * TOK], BF, tag="expt")
            if EXP_AS_COPY:
                nc.vector.tensor_copy(expt[:, 0:nkb * TOK],
                                      pscore[:, 0:nkb * TOK])
            else:
                nc.scalar.activation(expt[:, 0:nkb * TOK],
                                     pscore[:, 0:nkb * TOK], Exp, scale=SCALE)
            return expt

        def emit_av(state):
            h, kb0, nkb, expt, pav = state
            for i in range(nkb):
                kb = kb0 + i
                nc.tensor.matmul(
                    pav[:],
                    lhsT=vaug_all[:, kb, h, :],
                    rhs=expt[:, i * TOK:(i + 1) * TOK],
                    start=(kb == 0), stop=(kb == NKB - 1))

        def emit_normalize(h, pav):
            po, m = (h % 2) * 64, h // 2
            inv = smallp.tile([1, TOK], BF, tag="inv")
            with nc.allow_low_precision(reason="bf16 rounding of 1/sumexp"):
                nc.vector.reciprocal(inv[:], pav[64:65, :])
            pb = pbp.tile([64, TOK], F32, tag="pb")
            nc.tensor.matmul(pb[:], lhsT=ones[0:1, 0:64], rhs=inv[:],
                             start=True, stop=True)
            bcast = smallp.tile([64, TOK], F32, tag="bcast")
            nc.vector.tensor_copy(bcast[:], pb[:])
            nc.vector.tensor_mul(aout_sb[po:po + 64, m, :], pav[0:64, :],
                                 bcast[:])

        items = [(h, kb0, nkb)
                 for h in range(0 if SKIP_ATTN else 8)
                 for (kb0, nkb) in WAVES]
        pav_by_head = {}
        pending = None
        for (h, kb0, nkb) in items:
            if h not in pav_by_head:
                pav_by_head[h] = pavp.tile([65, TOK], F32, tag="av", name=f"pav{h}")
            pscore = emit_scores(h, kb0, nkb)
            if pending is not None:
                emit_av(pending)
                ph = pending[0]
                if ph != h:
                    emit_normalize(ph, pav_by_head.pop(ph))
            expt = emit_exp(pscore, nkb)
            pending = (h, kb0, nkb, expt, pav_by_head[h])
        if pending is not None:
            emit_av(pending)
            ph = pending[0]
            emit_normalize(ph, pav_by_head.pop(ph))
        pbp_cm.__exit__(None, None, None)
        pavp_cm.__exit__(None, None, None)
        pscorep_cm.__exit__(None, None, None)
        kvp_cm.__exit__(None, None, None)

        # ---- output projection + bias ----
        with tc.tile_pool(name="pworkc", bufs=2, space="PSUM") as pworkc:
            for a in range(4):
                for j in range(2):
                    py = pworkc.tile([128, 512], F32, tag="work")
                    for c in range(4):
                        nc.tensor.matmul(
                            py[:],
                            lhsT=aout_sb[:, c, a * 128:(a + 1) * 128],
                            rhs=wo_sb[:, c, j * 512:(j + 1) * 512],
                            start=(c == 0), stop=False)
                    nc.tensor.matmul(py[:], lhsT=ones[0:1, :],
                                     rhs=bo_sb[0:1, j * 512:(j + 1) * 512],
                                     start=False, stop=True)
                    yst = ysp.tile([128, 512], F32, tag="ys")
                    nc.vector.tensor_copy(yst[:], py[:])
                    nc.sync.dma_start(
                        y_d.ap()[a * 128:(a + 1) * 128,
                                 j * 512:(j + 1) * 512],
                        yst[:])


def _get_nc(reps=1):
    key = ("nc", NO_COLLECTIVE, SKIP_ATTN, SKIP_PROJ, EXP_AS_COPY, reps)
    if key not in _CACHE:
        _CACHE[key] = _build_kernel(no_collective=NO_COLLECTIVE, reps=reps)
    return _CACHE[key]


# ---------------------------------------------------------------------------
# Custom PJRT runner (mirrors bass2jax.run_bass_via_pjrt but builds the
# jitted executable once and keeps inputs device-resident so repeated calls
# measure device execution rather than host retrace/upload).
# ---------------------------------------------------------------------------

def _get_runner(reps=1):
    rkey = ("runner", NO_COLLECTIVE, SKIP_ATTN, SKIP_PROJ, EXP_AS_COPY, reps)
    if rkey in _CACHE:
        return _CACHE[rkey]
    import jax
    from jax.sharding import Mesh, PartitionSpec
    from jax.experimental.shard_map import shard_map
    from concourse import bass2jax as b2j
    import concourse.mybir as mb

    nc = _get_nc(reps)
    b2j.install_neuronx_cc_hook()

    partition_name = (nc.partition_id_tensor.name
                      if nc.partition_id_tensor else None)

    in_names, out_names, out_avals, zero_outs = [], [], [], []
    for alloc in nc.m.functions[0].allocations:
        if not isinstance(alloc, mb.MemoryLocationSet):
            continue
        name = alloc.memorylocations[0].name
        if alloc.kind == "ExternalInput":
            if name != partition_name:
                in_names.append(name)
        elif alloc.kind == "ExternalOutput":
            shape = tuple(alloc.tensor_shape)
            dtype = mb.dt.np(alloc.dtype)
            out_names.append(name)
            out_avals.append(jax.core.ShapedArray(shape, dtype))
            zero_outs.append(np.zeros(shape, dtype))
    n_params = len(in_names)
    all_names = in_names + out_names
    if partition_name is not None:
        all_names = all_names + [partition_name]

    def _body(*args):
        operands = list(args)
        if partition_name is not None:
            operands.append(b2j.partition_id_tensor())
        outs = b2j._bass_exec_p.bind(
            *operands,
            out_avals=tuple(out_avals),
            in_names=tuple(all_names),
            out_names=tuple(out_names),
            lowering_input_output_aliases=(),
            sim_require_finite=True,
            sim_require_nnan=True,
            nc=nc,
        )
        return tuple(outs)

    devices = jax.devices()[:N_CORES]
    mesh = Mesh(np.asarray(devices), ("core",))
    nin = n_params + len(out_names)

    def _once(*args):
        return _body(*args)

    x_idx = in_names.index("x_shard")

    donate = tuple(range(n_params, nin))

    def _make(nreps):
        def _fn(*args):
            ins = list(args[:n_params])
            zeros = list(args[n_params:])
            y = None
            for _ in range(nreps):
                outs = _body(*ins, *zeros)
                y = outs[0]
                ins[x_idx] = y
            return y
        return jax.jit(shard_map(
            _fn, mesh=mesh,
            in_specs=(PartitionSpec("core"),) * nin,
            out_specs=PartitionSpec("core"),
        ), donate_argnums=donate, keep_unused=True)

    run1 = jax.jit(shard_map(
        _once, mesh=mesh,
        in_specs=(PartitionSpec("core"),) * nin,
        out_specs=(PartitionSpec("core"),) * len(out_names),
    ), donate_argnums=donate, keep_unused=True)

    n_outs = len(out_names)

    def _make_multi(ncalls):
        # N independent executions per dispatch; each call gets its own zero
        # output buffers (distinct params defeat XLA CSE), no donation.
        def _fn(*args):
            ins = args[:n_params]
            ys = []
            for i in range(ncalls):
                zeros = args[n_params + i * n_outs:
                             n_params + (i + 1) * n_outs]
                outs = _body(*ins, *zeros)
                ys.append(outs[0])
            return tuple(ys)
        return jax.jit(shard_map(
            _fn, mesh=mesh,
            in_specs=(PartitionSpec("core"),) * (n_params + ncalls * n_outs),
            out_specs=(PartitionSpec("core"),) * ncalls,
        ), keep_unused=True)

    runner = {
        "run1": run1, "make": _make, "make_multi": _make_multi,
        "in_names": in_names,
        "out_names": out_names, "zero_outs": zero_outs,
        "n_params": n_params,
    }
    _CACHE[rkey] = runner
    return runner


def _device_args(in_maps):
    r = _get_runner()
    concat = [np.concatenate([in_maps[c][n] for c in range(N_CORES)], axis=0)
              for n in r["in_names"]]
    zeros = [np.zeros((N_CORES * z.shape[0], *z.shape[1:]), z.dtype)
             for z in r["zero_outs"]]
    return concat + zeros


def make_in_maps(x, Wq, Wkv, Wo, bo):
    x_flat = np.ascontiguousarray(
        np.asarray(x, dtype=np.float32).reshape(B * S, D))
    Wq = np.ascontiguousarray(np.asarray(Wq, dtype=np.float32))
    Wkv = np.ascontiguousarray(np.asarray(Wkv, dtype=np.float32))
    Wo = np.ascontiguousarray(np.asarray(Wo, dtype=np.float32))
    bo = np.ascontiguousarray(np.asarray(bo, dtype=np.float32))
    return [
        {"x_shard": np.ascontiguousarray(x_flat[c * TOK:(c + 1) * TOK]),
         "Wq": Wq, "Wkv": Wkv, "Wo": Wo, "bo": bo}
        for c in range(N_CORES)
    ]


def kernel(x, Wq, Wkv, Wo, bo):
    r = _get_runner()
    in_maps = make_in_maps(x, Wq, Wkv, Wo, bo)
    args = _device_args(in_maps)
    outs = r["run1"](*args)
    y = np.asarray(outs[0])
    return y.reshape(B, S, D).astype(np.float32)


def bench2(inputs, ncalls=17, nmeas=12):
    """Device-time estimate: N custom-call invocations inside one dispatch,
    fully blocked; compare medians of T(ncalls) vs T(1)."""
    import time
    import jax
    from jax.sharding import Mesh, PartitionSpec, NamedSharding
    r = _get_runner()
    n_params = r["n_params"]
    in_maps = make_in_maps(**inputs)
    base = _device_args(in_maps)
    devices = jax.devices()[:N_CORES]
    mesh = Mesh(np.asarray(devices), ("core",))
    shard = NamedSharding(mesh, PartitionSpec("core"))
    ins = [jax.device_put(a, shard) for a in base[:n_params]]
    zshapes = [a.shape for a in base[n_params:]]
    zsets = []
    for _ in range(ncalls):
        zsets.extend(jax.device_put(np.zeros(s, np.float32), shard)
                     for s in zshapes)
    jax.block_until_ready(ins)
    jax.block_until_ready(zsets)
    nz = len(zshapes)

    fn1 = r["make_multi"](1)
    fnN = r["make_multi"](ncalls)
    jax.block_until_ready(fn1(*ins, *zsets[:nz]))
    jax.block_until_ready(fnN(*ins, *zsets))

    def med(fn, a, n):
        ts = []
        for _ in range(n):
            t0 = time.perf_counter()
            jax.block_until_ready(fn(*a))
            ts.append(time.perf_counter() - t0)
        ts.sort()
        return ts[len(ts) // 2]

    t1 = med(fn1, ins + zsets[:nz], nmeas)
    tN = med(fnN, ins + zsets, nmeas)
    return (tN - t1) / (ncalls - 1), t1, tN


def bench3(inputs, reps=12, nmeas=12):
    """Device-time estimate via body repetition inside the NEFF."""
    import time
    import jax
    from jax.sharding import Mesh, PartitionSpec, NamedSharding

    def med_for(nreps):
        r = _get_runner(nreps)
        n_params = r["n_params"]
        in_maps = make_in_maps(**inputs)
        base = _device_args(in_maps)
        devices = jax.devices()[:N_CORES]
        mesh = Mesh(np.asarray(devices), ("core",))
        shard = NamedSharding(mesh, PartitionSpec("core"))
        ins = [jax.device_put(a, shard) for a in base[:n_params]]
        zshapes = [a.shape for a in base[n_params:]]
        fn = r["make_multi"](1)

        def mz():
            return [jax.device_put(np.zeros(s, np.float32), shard)
                    for s in zshapes]
        jax.block_until_ready(fn(*ins, *mz()))
        ts = []
        for _ in range(nmeas):
            zs = mz()
            jax.block_until_ready(zs)
            t0 = time.perf_counter()
            jax.block_until_ready(fn(*ins, *zs))
            ts.append(time.perf_counter() - t0)
        ts.sort()
        return ts[len(ts) // 2], ts

    t1, ts1 = med_for(1)
    tR, tsR = med_for(reps)
    per = (tR - t1) / (reps - 1)
    return per, t1, tR, ts1, tsR


def bench(inputs, nreps=10, nloops=3):
    """Return estimated per-execution wall time in seconds.

    Issues `nreps` async dispatches of the single-exec jit (device-resident
    inputs; fresh device-side zero buffers per call since outputs are
    donated), blocks once, and divides.
    """
    import time
    import jax
    import jax.numpy as jnp
    from jax.sharding import Mesh, PartitionSpec, NamedSharding
    r = _get_runner()
    n_params = r["n_params"]
    in_maps = make_in_maps(**inputs)
    base = _device_args(in_maps)

    devices = jax.devices()[:N_CORES]
    mesh = Mesh(np.asarray(devices), ("core",))
    shard = NamedSharding(mesh, PartitionSpec("core"))

    ins = [jax.device_put(a, shard) for a in base[:n_params]]
    zero_shapes = [a.shape for a in base[n_params:]]

    def make_zeros():
        zs = [jax.device_put(np.zeros(s, np.float32), shard)
              for s in zero_shapes]
        for z in zs:
            z.block_until_ready()
        return zs

    run1 = r["run1"]
    y = run1(*ins, *make_zeros())  # warm up / compile
    jax.block_until_ready(y)

    def run_batch(n):
        zsets = [make_zeros() for _ in range(n)]
        jax.block_until_ready(ins)
        t0 = time.perf_counter()
        ys = [run1(*ins, *zs) for zs in zsets]
        jax.block_until_ready(ys)
        return time.perf_counter() - t0

    n_lo, n_hi = nreps, 3 * nreps
    best = float("inf")
    for _ in range(nloops):
        t_lo = run_batch(n_lo)
        t_hi = run_batch(n_hi)
        slope = (t_hi - t_lo) / (n_hi - n_lo)
        best = min(best, slope)
    return best



# revision 13
# speedup vs baseline: 4.1189x; 4.1189x over previous
"""Multi-head self-attention (b=2, n=2048, d_model=1024, 8 heads x 64) on 8 TRN2 cores.

Sharding: token-parallel (512 tokens/core, batch-major). K and V are exchanged
via three 4-rank AllGathers (replica groups = batch element), split so compute
can start as soon as each piece lands:

  AG-K  : KT shard  [512 inner, 512 tok]  -> scores can start after this alone
  AG-V0 : V_aug shard heads 0-3 [512 tok, 4*65]
  AG-V1 : V_aug shard heads 4-7 [512 tok, 4*65]

All layouts avoid transposing the attention matrix:

  xT    [1024, 512]  (PE-transposed from x shard)
  QT/KT [512(inner), tokens] = W.T @ xT   (matmul lhsT=W chunk, rhs=xT chunk)
  V_aug [tokens, 8*(64+1)]   = xT.T @ Wv  (+ ones column per head)
  scoresT[keys,q]  = matmul(lhsT=KT[64,128], rhs=QT[64,512])
  expT   = ACT exp(0.125*scoresT)  PSUM->SBUF
  outT[65,q]      += matmul(lhsT=V_aug[128,65], rhs=expT[128,512])  (row 64 = sumexp)
  normalize via DVE reciprocal + K=1 broadcast matmul
  y[tok,1024]      = matmul(lhsT=aoutT[128,128], rhs=Wo[128,512]) + ones x bo

AV matmuls are emitted LAG waves behind the score/exp stream so the (strictly
FIFO) PE queue never stalls waiting for the V gathers to land.
"""

import numpy as np

import concourse.bass as bass
import concourse.mybir as mybir
import concourse.tile as tile
from concourse import bacc
from concourse.bass_utils import run_bass_kernel_spmd
from concourse.masks import make_identity

F32 = mybir.dt.float32
FR = mybir.dt.float32r
BF = mybir.dt.bfloat16

B, S, D = 2, 2048, 1024
H, DH = 8, 64
INNER = H * DH            # 512
N_CORES = 8
GROUP = 4                 # cores per batch element
TOK = (B * S) // N_CORES  # 512 tokens per core
NKB = S // 128            # 16 key blocks per batch context
SCALE = DH ** -0.5        # 0.125
WAVE = 2                  # key blocks per score/exp wave
NW = NKB // WAVE          # 8 waves per head
LAG = 22                  # AV emission lag, in waves
VW = 4 * 65               # 260: V_aug columns per V gather half

REPLICA_GROUPS = [[0, 1, 2, 3], [4, 5, 6, 7]]

_CACHE = {}
NO_COLLECTIVE = False   # timing A/B switch (wrong math, same local work)


def _build_kernel(no_collective=False, reps=1):
    nc = bacc.Bacc("TRN2", target_bir_lowering=False, debug=False,
                   num_devices=N_CORES)

    x_d = nc.dram_tensor("x_shard", [TOK, D], F32, kind="ExternalInput")
    wq_d = nc.dram_tensor("Wq", [D, INNER], F32, kind="ExternalInput")
    wkv_d = nc.dram_tensor("Wkv", [D, 2 * INNER], F32, kind="ExternalInput")
    wo_d = nc.dram_tensor("Wo", [INNER, D], F32, kind="ExternalInput")
    bo_d = nc.dram_tensor("bo", [D], F32, kind="ExternalInput")
    y_d = nc.dram_tensor("y_shard", [TOK, D], F32, kind="ExternalOutput")

    agk_in = nc.dram_tensor("agk_in", [INNER, TOK], BF, kind="Internal")
    agk_out = nc.dram_tensor("agk_out", [GROUP * INNER, TOK], BF,
                             kind="Internal")
    agv0_in = nc.dram_tensor("agv0_in", [TOK, VW], BF, kind="Internal")
    agv0_out = nc.dram_tensor("agv0_out", [GROUP * TOK, VW], BF,
                              kind="Internal")
    agv1_in = nc.dram_tensor("agv1_in", [TOK, VW], BF, kind="Internal")
    agv1_out = nc.dram_tensor("agv1_out", [GROUP * TOK, VW], BF,
                              kind="Internal")

    with tile.TileContext(nc) as tc:
        for _ in range(reps):
            _trace_body(nc, tc, x_d, wq_d, wkv_d, wo_d, bo_d, y_d,
                        agk_in, agk_out, agv0_in, agv0_out, agv1_in, agv1_out,
                        no_collective=no_collective)

    nc.compile()
    return nc


def _trace_body(nc, tc, x_d, wq_d, wkv_d, wo_d, bo_d, y_d,
                agk_in, agk_out, agv0_in, agv0_out, agv1_in, agv1_out,
                no_collective=False):
    Exp = mybir.ActivationFunctionType.Exp

    def fr(ap):
        # float32r: TF32-like PE mode, full-rate matmul at >=256 moving cols
        return ap.bitcast(mybir.dt.float32r)

    def all_gather(in_t, out_t, nrows):
        if no_collective:
            for r in range(GROUP):
                nc.sync.dma_start(
                    out_t.ap()[r * nrows:(r + 1) * nrows, :], in_t.ap())
        else:
            nc.gpsimd.collective_compute(
                "AllGather", mybir.AluOpType.bypass,
                replica_groups=REPLICA_GROUPS,
                ins=[in_t.ap()], outs=[out_t.ap()])

    with (
        tc.tile_pool(name="const", bufs=1) as constp,
        tc.tile_pool(name="wts", bufs=1) as wtsp,
        tc.tile_pool(name="qt", bufs=1) as qtp,
        tc.tile_pool(name="stage", bufs=3) as stagep,
        tc.tile_pool(name="expt", bufs=LAG + 3) as expp,
        tc.tile_pool(name="ao", bufs=1) as aop,
        tc.tile_pool(name="ys", bufs=2) as ysp,
        tc.tile_pool(name="small", bufs=2) as smallp,
    ):
        # ---- constants ----
        ident = constp.tile([128, 128], F32, tag="ident")
        make_identity(nc, ident[:])
        ones_f = constp.tile([1, 128], F32, tag="onesf")
        nc.gpsimd.memset(ones_f[:], 1.0)
        ones = constp.tile([1, 128], BF, tag="ones")
        nc.vector.tensor_copy(ones[:], ones_f[:])
        bo_sb = constp.tile([1, D], BF, tag="bo")
        nc.gpsimd.dma_start(
            bo_sb[:], bo_d.ap().rearrange("(a n) -> a n", a=1))

        # ---- persistent activations ----
        qt_sb = qtp.tile([128, 4, TOK], BF, tag="qt")          # QT [inner, tok]
        aout_sb = aop.tile([128, 4, TOK], BF, tag="aout")      # attnT out

        wo_sb = wtsp.tile([128, 4, D], BF, tag="wo")

        # ---- load x (two queues) and transpose: xT [1024, 512] ----
        wprojp_cm = tc.tile_pool(name="wproj", bufs=1)
        wprojp = wprojp_cm.__enter__()
        wkvk_sb = wprojp.tile([128, 8, INNER], BF, tag="wkvk")
        wkvv_sb = wprojp.tile([128, 8, INNER], BF, tag="wkvv")
        wq_sb = wprojp.tile([128, 8, INNER], BF, tag="wq")
        with (
            tc.tile_pool(name="xp", bufs=4) as xp,
            tc.tile_pool(name="xtp", bufs=1) as xtp,
            tc.tile_pool(name="pt", bufs=8, space="PSUM") as ptp,
        ):
            xt_sb = xtp.tile([128, 8, TOK], BF, tag="xt")
            x_tiles = []
            for a in range(4):
                x_t = xp.tile([128, D], F32, tag="x")
                eng = nc.sync if a < 2 else nc.scalar
                eng.dma_start(x_t[:], x_d.ap()[a * 128:(a + 1) * 128, :])
                x_tiles.append(x_t)

            # Weight loads (fp32->bf16 casting DMAs) must ride the
            # gpsimd queue; emitted BEFORE any collective so the triggers
            # fire before the gathers occupy the queue.
            nc.gpsimd.dma_start(
                wkvk_sb[:],
                wkv_d.ap()[:, 0:INNER].rearrange("(c p) n -> p c n", p=128))
            nc.gpsimd.dma_start(
                wkvv_sb[:],
                wkv_d.ap()[:, INNER:2 * INNER]
                .rearrange("(c p) n -> p c n", p=128))
            nc.gpsimd.dma_start(
                wq_sb[:], wq_d.ap().rearrange("(c p) n -> p c n", p=128))
            nc.gpsimd.dma_start(
                wo_sb[:], wo_d.ap().rearrange("(c p) n -> p c n", p=128))

            pts = [ptp.tile([128, TOK], F32, tag="pt", name=f"pt{c}")
                   for c in range(8)]
            for a in range(4):
                for c in range(8):
                    nc.tensor.transpose(
                        pts[c][:, a * 128:(a + 1) * 128],
                        x_tiles[a][:, c * 128:(c + 1) * 128], ident[:])
            for c in range(8):
                nc.vector.tensor_copy(xt_sb[:, c, :], pts[c][:])

        with tc.tile_pool(name="pworka", bufs=2, space="PSUM") as pworka:
            # ---- K projection -> agk_in, AllGather K ----
            for m in range(4):
                ps = pworka.tile([128, TOK], F32, tag="work")
                for c in range(8):
                    nc.tensor.matmul(ps[:],
                                     lhsT=wkvk_sb[:, c, m * 128:(m + 1) * 128],
                                     rhs=xt_sb[:, c, :],
                                     start=(c == 0), stop=(c == 7))
                st = stagep.tile([128, TOK], BF, tag="ktstage")
                nc.vector.tensor_copy(st[:], ps[:])
                nc.sync.dma_start(
                    agk_in.ap()[m * 128:(m + 1) * 128, :], st[:])
            all_gather(agk_in, agk_out, INNER)

            # ---- V projection (+ones col) -> agv0/agv1, AllGather V ----
            for a in range(4):
                ps = pworka.tile([128, INNER], F32, tag="work")
                for c in range(8):
                    nc.tensor.matmul(ps[:],
                                     lhsT=xt_sb[:, c, a * 128:(a + 1) * 128],
                                     rhs=wkvv_sb[:, c, :],
                                     start=(c == 0), stop=(c == 7))
                vst = stagep.tile([128, H, 65], BF, tag="vstage")
                nc.vector.tensor_copy(
                    vst[:, :, 0:64], ps[:].rearrange("p (h e) -> p h e", e=64))
                nc.vector.memset(vst[:, :, 64:65], 1.0)
                nc.sync.dma_start(
                    agv0_in.ap()[a * 128:(a + 1) * 128, :]
                    .rearrange("p (h e) -> p h e", e=65),
                    vst[:, 0:4, :])
                nc.sync.dma_start(
                    agv1_in.ap()[a * 128:(a + 1) * 128, :]
                    .rearrange("p (h e) -> p h e", e=65),
                    vst[:, 4:8, :])
            all_gather(agv0_in, agv0_out, TOK)
            all_gather(agv1_in, agv1_out, TOK)

            # ---- Q projection (runs under the K gather) ----
            for m in range(4):
                ps = pworka.tile([128, TOK], F32, tag="work")
                for c in range(8):
                    nc.tensor.matmul(ps[:],
                                     lhsT=wq_sb[:, c, m * 128:(m + 1) * 128],
                                     rhs=xt_sb[:, c, :],
                                     start=(c == 0), stop=(c == 7))
                nc.vector.tensor_copy(qt_sb[:, m, :], ps[:])

        wprojp_cm.__exit__(None, None, None)

        # ---- load gathered K/V ----
        kvp_cm = tc.tile_pool(name="kv", bufs=1)
        kvp = kvp_cm.__enter__()
        kt_all = kvp.tile([128, 4, GROUP, TOK], BF, tag="kt")  # p, m, r, t
        for r in range(GROUP):
            nc.sync.dma_start(
                kt_all[:, :, r, :],
                agk_out.ap()[r * INNER:(r + 1) * INNER, :]
                .rearrange("(m p) t -> p m t", p=128))
        vaug_lo = kvp.tile([128, NKB, 4, 65], BF, tag="vlo")
        nc.sync.dma_start(
            vaug_lo[:],
            agv0_out.ap().rearrange("(kb p) (h e) -> p kb h e", p=128, e=65))
        vaug_hi = kvp.tile([128, NKB, 4, 65], BF, tag="vhi")
        nc.sync.dma_start(
            vaug_hi[:],
            agv1_out.ap().rearrange("(kb p) (h e) -> p kb h e", p=128, e=65))

        # ---- attention, AV lagged by LAG waves ----
        pscorep_cm = tc.tile_pool(name="pscore", bufs=2, space="PSUM")
        pscorep = pscorep_cm.__enter__()
        pavp_cm = tc.tile_pool(name="pav", bufs=3, space="PSUM")
        pavp = pavp_cm.__enter__()
        pbp_cm = tc.tile_pool(name="pb", bufs=1, space="PSUM")
        pbp = pbp_cm.__enter__()

        def kt_slice(h, kb):
            po, m = (h % 2) * 64, h // 2
            return kt_all[po:po + 64, m, kb // 4,
                          (kb % 4) * 128:(kb % 4) * 128 + 128]

        def vaug_slice(h, kb):
            if h < 4:
                return vaug_lo[:, kb, h, :]
            return vaug_hi[:, kb, h - 4, :]

        def emit_scores(h, w):
            po, m = (h % 2) * 64, h // 2
            pscore = pscorep.tile([128, WAVE * TOK], F32, tag="s")
            for i in range(WAVE):
                kb = w * WAVE + i
                nc.tensor.matmul(
                    pscore[:, i * TOK:(i + 1) * TOK],
                    lhsT=kt_slice(h, kb),
                    rhs=qt_sb[po:po + 64, m, :],
                    start=True, stop=True)
            return pscore

        def emit_exp(pscore):
            expt = expp.tile([128, WAVE * TOK], BF, tag="expt")
            nc.scalar.activation(expt[:], pscore[:], Exp, scale=SCALE)
            return expt

        def emit_av(h, w, expt, pav):
            for i in range(WAVE):
                kb = w * WAVE + i
                nc.tensor.matmul(
                    pav[:],
                    lhsT=vaug_slice(h, kb),
                    rhs=expt[:, i * TOK:(i + 1) * TOK],
                    start=(kb == 0), stop=(kb == NKB - 1))

        def emit_normalize(h, pav):
            po, m = (h % 2) * 64, h // 2
            inv = smallp.tile([1, TOK], BF, tag="inv")
            with nc.allow_low_precision(reason="bf16 rounding of 1/sumexp"):
                nc.vector.reciprocal(inv[:], pav[64:65, :])
            pb = pbp.tile([64, TOK], F32, tag="pb")
            nc.tensor.matmul(pb[:], lhsT=ones[0:1, 0:64], rhs=inv[:],
                             start=True, stop=True)
            bcast = smallp.tile([64, TOK], F32, tag="bcast")
            nc.vector.tensor_copy(bcast[:], pb[:])
            nc.vector.tensor_mul(aout_sb[po:po + 64, m, :], pav[0:64, :],
                                 bcast[:])

        waves = [(h, w) for h in range(H) for w in range(NW)]
        pav_by_head = {}
        pending = []
        for g, (h, w) in enumerate(waves):
            pscore = emit_scores(h, w)
            expt = emit_exp(pscore)
            pending.append((h, w, expt))
            if g >= LAG:
                ph, pw, pexpt = pending[g - LAG]
                if ph not in pav_by_head:
                    pav_by_head[ph] = pavp.tile([65, TOK], F32, tag="av",
                                                name=f"pav{ph}")
                emit_av(ph, pw, pexpt, pav_by_head[ph])
                pending[g - LAG] = None
                if pw == NW - 1:
                    emit_normalize(ph, pav_by_head.pop(ph))
        for g in range(len(waves) - LAG, len(waves)):
            ph, pw, pexpt = pending[g]
            if ph not in pav_by_head:
                pav_by_head[ph] = pavp.tile([65, TOK], F32, tag="av",
                                            name=f"pav{ph}")
            emit_av(ph, pw, pexpt, pav_by_head[ph])
            if pw == NW - 1:
                emit_normalize(ph, pav_by_head.pop(ph))
        pbp_cm.__exit__(None, None, None)
        pavp_cm.__exit__(None, None, None)
        pscorep_cm.__exit__(None, None, None)
        kvp_cm.__exit__(None, None, None)

        # ---- output projection + bias ----
        with tc.tile_pool(name="pworkc", bufs=2, space="PSUM") as pworkc:
            for a in range(4):
                for j in range(2):
                    py = pworkc.tile([128, 512], F32, tag="work")
                    for c in range(4):
                        nc.tensor.matmul(
                            py[:],
                            lhsT=aout_sb[:, c, a * 128:(a + 1) * 128],
                            rhs=wo_sb[:, c, j * 512:(j + 1) * 512],
                            start=(c == 0), stop=False)
                    nc.tensor.matmul(py[:], lhsT=ones[0:1, :],
                                     rhs=bo_sb[0:1, j * 512:(j + 1) * 512],
                                     start=False, stop=True)
                    yst = ysp.tile([128, 512], F32, tag="ys")
                    nc.vector.tensor_copy(yst[:], py[:])
                    nc.sync.dma_start(
                        y_d.ap()[a * 128:(a + 1) * 128,
                                 j * 512:(j + 1) * 512],
                        yst[:])


def _get_nc(reps=1):
    key = ("nc", NO_COLLECTIVE, reps)
    if key not in _CACHE:
        _CACHE[key] = _build_kernel(no_collective=NO_COLLECTIVE, reps=reps)
    return _CACHE[key]


# ---------------------------------------------------------------------------
# Custom PJRT runner (mirrors bass2jax.run_bass_via_pjrt but builds the
# jitted executable once and keeps inputs device-resident so repeated calls
# measure device execution rather than host retrace/upload).
# ---------------------------------------------------------------------------

def _get_runner(reps=1):
    rkey = ("runner", NO_COLLECTIVE, reps)
    if rkey in _CACHE:
        return _CACHE[rkey]
    import jax
    from jax.sharding import Mesh, PartitionSpec
    from jax.experimental.shard_map import shard_map
    from concourse import bass2jax as b2j
    import concourse.mybir as mb

    nc = _get_nc(reps)
    b2j.install_neuronx_cc_hook()

    partition_name = (nc.partition_id_tensor.name
                      if nc.partition_id_tensor else None)

    in_names, out_names, out_avals, zero_outs = [], [], [], []
    for alloc in nc.m.functions[0].allocations:
        if not isinstance(alloc, mb.MemoryLocationSet):
            continue
        name = alloc.memorylocations[0].name
        if alloc.kind == "ExternalInput":
            if name != partition_name:
                in_names.append(name)
        elif alloc.kind == "ExternalOutput":
            shape = tuple(alloc.tensor_shape)
            dtype = mb.dt.np(alloc.dtype)
            out_names.append(name)
            out_avals.append(jax.core.ShapedArray(shape, dtype))
            zero_outs.append(np.zeros(shape, dtype))
    n_params = len(in_names)
    all_names = in_names + out_names
    if partition_name is not None:
        all_names = all_names + [partition_name]

    def _body(*args):
        operands = list(args)
        if partition_name is not None:
            operands.append(b2j.partition_id_tensor())
        outs = b2j._bass_exec_p.bind(
            *operands,
            out_avals=tuple(out_avals),
            in_names=tuple(all_names),
            out_names=tuple(out_names),
            lowering_input_output_aliases=(),
            sim_require_finite=True,
            sim_require_nnan=True,
            nc=nc,
        )
        return tuple(outs)

    devices = jax.devices()[:N_CORES]
    mesh = Mesh(np.asarray(devices), ("core",))
    nin = n_params + len(out_names)

    def _once(*args):
        return _body(*args)

    donate = tuple(range(n_params, nin))

    run1 = jax.jit(shard_map(
        _once, mesh=mesh,
        in_specs=(PartitionSpec("core"),) * nin,
        out_specs=(PartitionSpec("core"),) * len(out_names),
    ), donate_argnums=donate, keep_unused=True)

    n_outs = len(out_names)

    def _make_multi(ncalls):
        # N independent executions per dispatch; each call gets its own zero
        # output buffers (distinct params defeat XLA CSE), no donation.
        def _fn(*args):
            ins = args[:n_params]
            ys = []
            for i in range(ncalls):
                zeros = args[n_params + i * n_outs:
                             n_params + (i + 1) * n_outs]
                outs = _body(*ins, *zeros)
                ys.append(outs[0])
            return tuple(ys)
        return jax.jit(shard_map(
            _fn, mesh=mesh,
            in_specs=(PartitionSpec("core"),) * (n_params + ncalls * n_outs),
            out_specs=(PartitionSpec("core"),) * ncalls,
        ), keep_unused=True)

    runner = {
        "run1": run1, "make_multi": _make_multi,
        "in_names": in_names,
        "out_names": out_names, "zero_outs": zero_outs,
        "n_params": n_params,
    }
    _CACHE[rkey] = runner
    return runner


def _device_args(in_maps, reps=1):
    r = _get_runner(reps)
    concat = [np.concatenate([in_maps[c][n] for c in range(N_CORES)], axis=0)
              for n in r["in_names"]]
    zeros = [np.zeros((N_CORES * z.shape[0], *z.shape[1:]), z.dtype)
             for z in r["zero_outs"]]
    return concat + zeros


def make_in_maps(x, Wq, Wkv, Wo, bo):
    x_flat = np.ascontiguousarray(
        np.asarray(x, dtype=np.float32).reshape(B * S, D))
    Wq = np.ascontiguousarray(np.asarray(Wq, dtype=np.float32))
    Wkv = np.ascontiguousarray(np.asarray(Wkv, dtype=np.float32))
    Wo = np.ascontiguousarray(np.asarray(Wo, dtype=np.float32))
    bo = np.ascontiguousarray(np.asarray(bo, dtype=np.float32))
    return [
        {"x_shard": np.ascontiguousarray(x_flat[c * TOK:(c + 1) * TOK]),
         "Wq": Wq, "Wkv": Wkv, "Wo": Wo, "bo": bo}
        for c in range(N_CORES)
    ]


def kernel(x, Wq, Wkv, Wo, bo):
    r = _get_runner()
    in_maps = make_in_maps(x, Wq, Wkv, Wo, bo)
    args = _device_args(in_maps)
    outs = r["run1"](*args)
    y = np.asarray(outs[0])
    return y.reshape(B, S, D).astype(np.float32)


def bench3(inputs, reps=24, nmeas=12, lo_reps=1):
    """Per-exec device time via body repetition inside the NEFF: interleaved
    measurements of T(lo_reps) and T(reps); slope from median of differences."""
    import time
    import jax
    from jax.sharding import Mesh, PartitionSpec, NamedSharding

    devices = jax.devices()[:N_CORES]
    mesh = Mesh(np.asarray(devices), ("core",))
    shard = NamedSharding(mesh, PartitionSpec("core"))
    in_maps = make_in_maps(**inputs)

    def prep(nreps):
        r = _get_runner(nreps)
        base = _device_args(in_maps, nreps)
        n_params = r["n_params"]
        ins = [jax.device_put(a, shard) for a in base[:n_params]]
        zshapes = [a.shape for a in base[n_params:]]
        fn = r["make_multi"](1)

        def mz():
            return [jax.device_put(np.zeros(s, np.float32), shard)
                    for s in zshapes]
        jax.block_until_ready(fn(*ins, *mz()))  # warm / compile
        return fn, ins, mz

    fn_lo, ins_lo, mz_lo = prep(lo_reps)
    fn_hi, ins_hi, mz_hi = prep(reps)

    def timed(fn, ins, mz):
        zs = mz()
        jax.block_until_ready(zs)
        t0 = time.perf_counter()
        jax.block_until_ready(fn(*ins, *zs))
        return time.perf_counter() - t0

    diffs, los, his = [], [], []
    for _ in range(nmeas):
        tl = timed(fn_lo, ins_lo, mz_lo)
        th = timed(fn_hi, ins_hi, mz_hi)
        diffs.append(th - tl)
        los.append(tl)
        his.append(th)
    diffs.sort()
    med = diffs[len(diffs) // 2]
    per = med / (reps - lo_reps)
    return per, los, his


def bench(inputs, nreps=10, nloops=3):
    """Return estimated per-execution wall time in seconds (chained async
    dispatches; includes per-dispatch host/tunnel overhead)."""
    import time
    import jax
    from jax.sharding import Mesh, PartitionSpec, NamedSharding
    r = _get_runner()
    n_params = r["n_params"]
    in_maps = make_in_maps(**inputs)
    base = _device_args(in_maps)

    devices = jax.devices()[:N_CORES]
    mesh = Mesh(np.asarray(devices), ("core",))
    shard = NamedSharding(mesh, PartitionSpec("core"))

    ins = [jax.device_put(a, shard) for a in base[:n_params]]
    zero_shapes = [a.shape for a in base[n_params:]]

    def make_zeros():
        zs = [jax.device_put(np.zeros(s, np.float32), shard)
              for s in zero_shapes]
        for z in zs:
            z.block_until_ready()
        return zs

    run1 = r["run1"]
    y = run1(*ins, *make_zeros())  # warm up / compile
    jax.block_until_ready(y)

    def run_batch(n):
        zsets = [make_zeros() for _ in range(n)]
        jax.block_until_ready(ins)
        t0 = time.perf_counter()
        ys = [run1(*ins, *zs) for zs in zsets]
        jax.block_until_ready(ys)
        return time.perf_counter() - t0

    n_lo, n_hi = nreps, 3 * nreps
    best = float("inf")
    for _ in range(nloops):
        t_lo = run_batch(n_lo)
        t_hi = run_batch(n_hi)
        slope = (t_hi - t_lo) / (n_hi - n_lo)
        best = min(best, slope)
    return best


# revision 14
# speedup vs baseline: 4.3306x; 1.0514x over previous
"""Multi-head self-attention (b=2, n=2048, d_model=1024, 8 heads x 64) on 8 TRN2 cores.

Sharding: token-parallel (512 tokens/core, batch-major). K and V are exchanged
via three 4-rank AllGathers (replica groups = batch element), split so compute
can start as soon as each piece lands:

  AG-K  : KT shard  [512 inner, 512 tok]  -> scores can start after this alone
  AG-V0 : V_aug shard heads 0-3 [512 tok, 4*65]
  AG-V1 : V_aug shard heads 4-7 [512 tok, 4*65]

All layouts avoid transposing the attention matrix:

  xT    [1024, 512]  (PE-transposed from x shard)
  QT/KT [512(inner), tokens] = W.T @ xT   (matmul lhsT=W chunk, rhs=xT chunk)
  V_aug [tokens, 8*(64+1)]   = xT.T @ Wv  (+ ones column per head)
  scoresT[keys,q]  = matmul(lhsT=KT[64,128], rhs=QT[64,512])
  expT   = ACT exp(0.125*scoresT)  PSUM->SBUF
  outT[65,q]      += matmul(lhsT=V_aug[128,65], rhs=expT[128,512])  (row 64 = sumexp)
  normalize via DVE reciprocal + K=1 broadcast matmul
  y[tok,1024]      = matmul(lhsT=aoutT[128,128], rhs=Wo[128,512]) + ones x bo

AV matmuls are emitted LAG waves behind the score/exp stream so the (strictly
FIFO) PE queue never stalls waiting for the V gathers to land.
"""

import numpy as np

import concourse.bass as bass
import concourse.mybir as mybir
import concourse.tile as tile
from concourse import bacc
from concourse.bass_utils import run_bass_kernel_spmd
from concourse.masks import make_identity

F32 = mybir.dt.float32
FR = mybir.dt.float32r
BF = mybir.dt.bfloat16

B, S, D = 2, 2048, 1024
H, DH = 8, 64
INNER = H * DH            # 512
N_CORES = 8
GROUP = 4                 # cores per batch element
TOK = (B * S) // N_CORES  # 512 tokens per core
NKB = S // 128            # 16 key blocks per batch context
SCALE = DH ** -0.5        # 0.125
WAVE = 2                  # key blocks per score/exp wave
NW = NKB // WAVE          # 8 waves per head
LAG = 16                  # AV emission lag, in waves
VW = 4 * 65               # 260: V_aug columns per V gather half

REPLICA_GROUPS = [[0, 1, 2, 3], [4, 5, 6, 7]]

_CACHE = {}
NO_COLLECTIVE = False   # timing A/B switch (wrong math, same local work)


def _build_kernel(no_collective=False, reps=1):
    nc = bacc.Bacc("TRN2", target_bir_lowering=False, debug=False,
                   num_devices=N_CORES)

    x_d = nc.dram_tensor("x_shard", [TOK, D], F32, kind="ExternalInput")
    wq_d = nc.dram_tensor("Wq", [D, INNER], F32, kind="ExternalInput")
    wkv_d = nc.dram_tensor("Wkv", [D, 2 * INNER], F32, kind="ExternalInput")
    wo_d = nc.dram_tensor("Wo", [INNER, D], F32, kind="ExternalInput")
    bo_d = nc.dram_tensor("bo", [D], F32, kind="ExternalInput")
    y_d = nc.dram_tensor("y_shard", [TOK, D], F32, kind="ExternalOutput")

    agk_in = nc.dram_tensor("agk_in", [INNER, TOK], BF, kind="Internal")
    agk_out = nc.dram_tensor("agk_out", [GROUP * INNER, TOK], BF,
                             kind="Internal")
    agv0_in = nc.dram_tensor("agv0_in", [TOK, VW], BF, kind="Internal")
    agv0_out = nc.dram_tensor("agv0_out", [GROUP * TOK, VW], BF,
                              kind="Internal")
    agv1_in = nc.dram_tensor("agv1_in", [TOK, VW], BF, kind="Internal")
    agv1_out = nc.dram_tensor("agv1_out", [GROUP * TOK, VW], BF,
                              kind="Internal")

    with tile.TileContext(nc) as tc:
        for _ in range(reps):
            _trace_body(nc, tc, x_d, wq_d, wkv_d, wo_d, bo_d, y_d,
                        agk_in, agk_out, agv0_in, agv0_out, agv1_in, agv1_out,
                        no_collective=no_collective)

    nc.compile()
    return nc


def _trace_body(nc, tc, x_d, wq_d, wkv_d, wo_d, bo_d, y_d,
                agk_in, agk_out, agv0_in, agv0_out, agv1_in, agv1_out,
                no_collective=False):
    Exp = mybir.ActivationFunctionType.Exp

    def fr(ap):
        # float32r: TF32-like PE mode, full-rate matmul at >=256 moving cols
        return ap.bitcast(mybir.dt.float32r)

    def all_gather(in_t, out_t, nrows):
        if no_collective:
            for r in range(GROUP):
                nc.sync.dma_start(
                    out_t.ap()[r * nrows:(r + 1) * nrows, :], in_t.ap())
        else:
            nc.gpsimd.collective_compute(
                "AllGather", mybir.AluOpType.bypass,
                replica_groups=REPLICA_GROUPS,
                ins=[in_t.ap()], outs=[out_t.ap()])

    with (
        tc.tile_pool(name="const", bufs=1) as constp,
        tc.tile_pool(name="wts", bufs=1) as wtsp,
        tc.tile_pool(name="qt", bufs=1) as qtp,
        tc.tile_pool(name="stage", bufs=3) as stagep,
        tc.tile_pool(name="expt", bufs=LAG + 3) as expp,
        tc.tile_pool(name="ao", bufs=1) as aop,
        tc.tile_pool(name="ys", bufs=2) as ysp,
        tc.tile_pool(name="small", bufs=2) as smallp,
    ):
        # ---- constants ----
        ident = constp.tile([128, 128], F32, tag="ident")
        make_identity(nc, ident[:])
        ones_f = constp.tile([1, 128], F32, tag="onesf")
        nc.gpsimd.memset(ones_f[:], 1.0)
        ones = constp.tile([1, 128], BF, tag="ones")
        nc.vector.tensor_copy(ones[:], ones_f[:])
        bo_sb = constp.tile([1, D], BF, tag="bo")
        nc.gpsimd.dma_start(
            bo_sb[:], bo_d.ap().rearrange("(a n) -> a n", a=1))

        # ---- persistent activations ----
        qt_sb = qtp.tile([128, 4, TOK], BF, tag="qt")          # QT [inner, tok]
        aout_sb = aop.tile([128, 4, TOK], BF, tag="aout")      # attnT out

        wo_sb = wtsp.tile([128, 4, D], BF, tag="wo")

        # ---- load x (two queues) and transpose: xT [1024, 512] ----
        wprojp_cm = tc.tile_pool(name="wproj", bufs=1)
        wprojp = wprojp_cm.__enter__()
        wkvk_sb = wprojp.tile([128, 8, INNER], BF, tag="wkvk")
        wkvv_sb = wprojp.tile([128, 8, INNER], BF, tag="wkvv")
        wq_sb = wprojp.tile([128, 8, INNER], BF, tag="wq")
        with (
            tc.tile_pool(name="xp", bufs=4) as xp,
            tc.tile_pool(name="xtp", bufs=1) as xtp,
            tc.tile_pool(name="pt", bufs=8, space="PSUM") as ptp,
        ):
            xt_sb = xtp.tile([128, 8, TOK], BF, tag="xt")
            x_tiles = []
            for a in range(4):
                x_t = xp.tile([128, D], F32, tag="x")
                eng = nc.sync if a < 2 else nc.scalar
                eng.dma_start(x_t[:], x_d.ap()[a * 128:(a + 1) * 128, :])
                x_tiles.append(x_t)

            # Weight loads (fp32->bf16 casting DMAs) must ride the
            # gpsimd queue; emitted BEFORE any collective so the triggers
            # fire before the gathers occupy the queue.
            nc.gpsimd.dma_start(
                wkvk_sb[:],
                wkv_d.ap()[:, 0:INNER].rearrange("(c p) n -> p c n", p=128))
            nc.gpsimd.dma_start(
                wkvv_sb[:],
                wkv_d.ap()[:, INNER:2 * INNER]
                .rearrange("(c p) n -> p c n", p=128))
            nc.gpsimd.dma_start(
                wq_sb[:], wq_d.ap().rearrange("(c p) n -> p c n", p=128))
            nc.gpsimd.dma_start(
                wo_sb[:], wo_d.ap().rearrange("(c p) n -> p c n", p=128))

            pts = [ptp.tile([128, TOK], F32, tag="pt", name=f"pt{c}")
                   for c in range(8)]
            for a in range(4):
                for c in range(8):
                    nc.tensor.transpose(
                        pts[c][:, a * 128:(a + 1) * 128],
                        x_tiles[a][:, c * 128:(c + 1) * 128], ident[:])
            for c in range(8):
                nc.vector.tensor_copy(xt_sb[:, c, :], pts[c][:])

        with tc.tile_pool(name="pworka", bufs=2, space="PSUM") as pworka:
            # ---- K projection -> agk_in, AllGather K ----
            for m in range(4):
                ps = pworka.tile([128, TOK], F32, tag="work")
                for c in range(8):
                    nc.tensor.matmul(ps[:],
                                     lhsT=wkvk_sb[:, c, m * 128:(m + 1) * 128],
                                     rhs=xt_sb[:, c, :],
                                     start=(c == 0), stop=(c == 7))
                st = stagep.tile([128, TOK], BF, tag="ktstage")
                nc.vector.tensor_copy(st[:], ps[:])
                nc.sync.dma_start(
                    agk_in.ap()[m * 128:(m + 1) * 128, :], st[:])
            all_gather(agk_in, agk_out, INNER)

            # ---- V projection (+ones col) -> agv0/agv1, AllGather V ----
            for a in range(4):
                ps = pworka.tile([128, INNER], F32, tag="work")
                for c in range(8):
                    nc.tensor.matmul(ps[:],
                                     lhsT=xt_sb[:, c, a * 128:(a + 1) * 128],
                                     rhs=wkvv_sb[:, c, :],
                                     start=(c == 0), stop=(c == 7))
                vst = stagep.tile([128, H, 65], BF, tag="vstage")
                nc.vector.tensor_copy(
                    vst[:, :, 0:64], ps[:].rearrange("p (h e) -> p h e", e=64))
                nc.vector.memset(vst[:, :, 64:65], 1.0)
                nc.sync.dma_start(
                    agv0_in.ap()[a * 128:(a + 1) * 128, :]
                    .rearrange("p (h e) -> p h e", e=65),
                    vst[:, 0:4, :])
                nc.sync.dma_start(
                    agv1_in.ap()[a * 128:(a + 1) * 128, :]
                    .rearrange("p (h e) -> p h e", e=65),
                    vst[:, 4:8, :])
            all_gather(agv0_in, agv0_out, TOK)
            all_gather(agv1_in, agv1_out, TOK)

            # ---- Q projection (runs under the K gather) ----
            for m in range(4):
                ps = pworka.tile([128, TOK], F32, tag="work")
                for c in range(8):
                    nc.tensor.matmul(ps[:],
                                     lhsT=wq_sb[:, c, m * 128:(m + 1) * 128],
                                     rhs=xt_sb[:, c, :],
                                     start=(c == 0), stop=(c == 7))
                nc.vector.tensor_copy(qt_sb[:, m, :], ps[:])

        wprojp_cm.__exit__(None, None, None)

        # ---- load gathered K/V ----
        kvp_cm = tc.tile_pool(name="kv", bufs=1)
        kvp = kvp_cm.__enter__()
        kt_all = kvp.tile([128, 4, GROUP, TOK], BF, tag="kt")  # p, m, r, t
        for r in range(GROUP):
            nc.sync.dma_start(
                kt_all[:, :, r, :],
                agk_out.ap()[r * INNER:(r + 1) * INNER, :]
                .rearrange("(m p) t -> p m t", p=128))
        vaug_lo = kvp.tile([128, NKB, 4, 65], BF, tag="vlo")
        nc.sync.dma_start(
            vaug_lo[:],
            agv0_out.ap().rearrange("(kb p) (h e) -> p kb h e", p=128, e=65))
        vaug_hi = kvp.tile([128, NKB, 4, 65], BF, tag="vhi")
        nc.sync.dma_start(
            vaug_hi[:],
            agv1_out.ap().rearrange("(kb p) (h e) -> p kb h e", p=128, e=65))

        # ---- attention, AV lagged by LAG waves ----
        pscorep_cm = tc.tile_pool(name="pscore", bufs=2, space="PSUM")
        pscorep = pscorep_cm.__enter__()
        pavp_cm = tc.tile_pool(name="pav", bufs=3, space="PSUM")
        pavp = pavp_cm.__enter__()
        pbp_cm = tc.tile_pool(name="pb", bufs=1, space="PSUM")
        pbp = pbp_cm.__enter__()

        def kt_slice(h, kb):
            po, m = (h % 2) * 64, h // 2
            return kt_all[po:po + 64, m, kb // 4,
                          (kb % 4) * 128:(kb % 4) * 128 + 128]

        def vaug_slice(h, kb):
            if h < 4:
                return vaug_lo[:, kb, h, :]
            return vaug_hi[:, kb, h - 4, :]

        def emit_scores(h, w):
            po, m = (h % 2) * 64, h // 2
            pscore = pscorep.tile([128, WAVE * TOK], F32, tag="s")
            for i in range(WAVE):
                kb = w * WAVE + i
                nc.tensor.matmul(
                    pscore[:, i * TOK:(i + 1) * TOK],
                    lhsT=kt_slice(h, kb),
                    rhs=qt_sb[po:po + 64, m, :],
                    start=True, stop=True)
            return pscore

        def emit_exp(pscore):
            expt = expp.tile([128, WAVE * TOK], BF, tag="expt")
            nc.scalar.activation(expt[:], pscore[:], Exp, scale=SCALE)
            return expt

        def emit_av(h, w, expt, pav):
            for i in range(WAVE):
                kb = w * WAVE + i
                nc.tensor.matmul(
                    pav[:],
                    lhsT=vaug_slice(h, kb),
                    rhs=expt[:, i * TOK:(i + 1) * TOK],
                    start=(kb == 0), stop=(kb == NKB - 1))

        def emit_normalize(h, pav):
            po, m = (h % 2) * 64, h // 2
            inv = smallp.tile([1, TOK], BF, tag="inv")
            with nc.allow_low_precision(reason="bf16 rounding of 1/sumexp"):
                nc.vector.reciprocal(inv[:], pav[64:65, :])
            pb = pbp.tile([64, TOK], F32, tag="pb")
            nc.tensor.matmul(pb[:], lhsT=ones[0:1, 0:64], rhs=inv[:],
                             start=True, stop=True)
            bcast = smallp.tile([64, TOK], F32, tag="bcast")
            nc.vector.tensor_copy(bcast[:], pb[:])
            nc.vector.tensor_mul(aout_sb[po:po + 64, m, :], pav[0:64, :],
                                 bcast[:])

        waves = [(h, w) for h in range(H) for w in range(NW)]
        pav_by_head = {}
        pending = []
        for g, (h, w) in enumerate(waves):
            pscore = emit_scores(h, w)
            expt = emit_exp(pscore)
            pending.append((h, w, expt))
            if g >= LAG:
                ph, pw, pexpt = pending[g - LAG]
                if ph not in pav_by_head:
                    pav_by_head[ph] = pavp.tile([65, TOK], F32, tag="av",
                                                name=f"pav{ph}")
                emit_av(ph, pw, pexpt, pav_by_head[ph])
                pending[g - LAG] = None
                if pw == NW - 1:
                    emit_normalize(ph, pav_by_head.pop(ph))
        for g in range(len(waves) - LAG, len(waves)):
            ph, pw, pexpt = pending[g]
            if ph not in pav_by_head:
                pav_by_head[ph] = pavp.tile([65, TOK], F32, tag="av",
                                            name=f"pav{ph}")
            emit_av(ph, pw, pexpt, pav_by_head[ph])
            if pw == NW - 1:
                emit_normalize(ph, pav_by_head.pop(ph))
        pbp_cm.__exit__(None, None, None)
        pavp_cm.__exit__(None, None, None)
        pscorep_cm.__exit__(None, None, None)
        kvp_cm.__exit__(None, None, None)

        # ---- output projection + bias ----
        with tc.tile_pool(name="pworkc", bufs=2, space="PSUM") as pworkc:
            for a in range(4):
                for j in range(2):
                    py = pworkc.tile([128, 512], F32, tag="work")
                    for c in range(4):
                        nc.tensor.matmul(
                            py[:],
                            lhsT=aout_sb[:, c, a * 128:(a + 1) * 128],
                            rhs=wo_sb[:, c, j * 512:(j + 1) * 512],
                            start=(c == 0), stop=False)
                    nc.tensor.matmul(py[:], lhsT=ones[0:1, :],
                                     rhs=bo_sb[0:1, j * 512:(j + 1) * 512],
                                     start=False, stop=True)
                    yst = ysp.tile([128, 512], F32, tag="ys")
                    nc.vector.tensor_copy(yst[:], py[:])
                    nc.sync.dma_start(
                        y_d.ap()[a * 128:(a + 1) * 128,
                                 j * 512:(j + 1) * 512],
                        yst[:])


def _get_nc(reps=1):
    key = ("nc", NO_COLLECTIVE, reps)
    if key not in _CACHE:
        _CACHE[key] = _build_kernel(no_collective=NO_COLLECTIVE, reps=reps)
    return _CACHE[key]


# ---------------------------------------------------------------------------
# Custom PJRT runner (mirrors bass2jax.run_bass_via_pjrt but builds the
# jitted executable once and keeps inputs device-resident so repeated calls
# measure device execution rather than host retrace/upload).
# ---------------------------------------------------------------------------

def _get_runner(reps=1):
    rkey = ("runner", NO_COLLECTIVE, reps)
    if rkey in _CACHE:
        return _CACHE[rkey]
    import jax
    from jax.sharding import Mesh, PartitionSpec
    from jax.experimental.shard_map import shard_map
    from concourse import bass2jax as b2j
    import concourse.mybir as mb

    nc = _get_nc(reps)
    b2j.install_neuronx_cc_hook()

    partition_name = (nc.partition_id_tensor.name
                      if nc.partition_id_tensor else None)

    in_names, out_names, out_avals, zero_outs = [], [], [], []
    for alloc in nc.m.functions[0].allocations:
        if not isinstance(alloc, mb.MemoryLocationSet):
            continue
        name = alloc.memorylocations[0].name
        if alloc.kind == "ExternalInput":
            if name != partition_name:
                in_names.append(name)
        elif alloc.kind == "ExternalOutput":
            shape = tuple(alloc.tensor_shape)
            dtype = mb.dt.np(alloc.dtype)
            out_names.append(name)
            out_avals.append(jax.core.ShapedArray(shape, dtype))
            zero_outs.append(np.zeros(shape, dtype))
    n_params = len(in_names)
    all_names = in_names + out_names
    if partition_name is not None:
        all_names = all_names + [partition_name]

    def _body(*args):
        operands = list(args)
        if partition_name is not None:
            operands.append(b2j.partition_id_tensor())
        outs = b2j._bass_exec_p.bind(
            *operands,
            out_avals=tuple(out_avals),
            in_names=tuple(all_names),
            out_names=tuple(out_names),
            lowering_input_output_aliases=(),
            sim_require_finite=True,
            sim_require_nnan=True,
            nc=nc,
        )
        return tuple(outs)

    devices = jax.devices()[:N_CORES]
    mesh = Mesh(np.asarray(devices), ("core",))
    nin = n_params + len(out_names)

    def _once(*args):
        return _body(*args)

    donate = tuple(range(n_params, nin))

    run1 = jax.jit(shard_map(
        _once, mesh=mesh,
        in_specs=(PartitionSpec("core"),) * nin,
        out_specs=(PartitionSpec("core"),) * len(out_names),
    ), donate_argnums=donate, keep_unused=True)

    n_outs = len(out_names)

    def _make_multi(ncalls):
        # N independent executions per dispatch; each call gets its own zero
        # output buffers (distinct params defeat XLA CSE), no donation.
        def _fn(*args):
            ins = args[:n_params]
            ys = []
            for i in range(ncalls):
                zeros = args[n_params + i * n_outs:
                             n_params + (i + 1) * n_outs]
                outs = _body(*ins, *zeros)
                ys.append(outs[0])
            return tuple(ys)
        return jax.jit(shard_map(
            _fn, mesh=mesh,
            in_specs=(PartitionSpec("core"),) * (n_params + ncalls * n_outs),
            out_specs=(PartitionSpec("core"),) * ncalls,
        ), keep_unused=True)

    runner = {
        "run1": run1, "make_multi": _make_multi,
        "in_names": in_names,
        "out_names": out_names, "zero_outs": zero_outs,
        "n_params": n_params,
    }
    _CACHE[rkey] = runner
    return runner


def _device_args(in_maps, reps=1):
    r = _get_runner(reps)
    concat = [np.concatenate([in_maps[c][n] for c in range(N_CORES)], axis=0)
              for n in r["in_names"]]
    zeros = [np.zeros((N_CORES * z.shape[0], *z.shape[1:]), z.dtype)
             for z in r["zero_outs"]]
    return concat + zeros


def make_in_maps(x, Wq, Wkv, Wo, bo):
    x_flat = np.ascontiguousarray(
        np.asarray(x, dtype=np.float32).reshape(B * S, D))
    Wq = np.ascontiguousarray(np.asarray(Wq, dtype=np.float32))
    Wkv = np.ascontiguousarray(np.asarray(Wkv, dtype=np.float32))
    Wo = np.ascontiguousarray(np.asarray(Wo, dtype=np.float32))
    bo = np.ascontiguousarray(np.asarray(bo, dtype=np.float32))
    return [
        {"x_shard": np.ascontiguousarray(x_flat[c * TOK:(c + 1) * TOK]),
         "Wq": Wq, "Wkv": Wkv, "Wo": Wo, "bo": bo}
        for c in range(N_CORES)
    ]


def kernel(x, Wq, Wkv, Wo, bo):
    r = _get_runner()
    in_maps = make_in_maps(x, Wq, Wkv, Wo, bo)
    args = _device_args(in_maps)
    outs = r["run1"](*args)
    y = np.asarray(outs[0])
    return y.reshape(B, S, D).astype(np.float32)


def bench3(inputs, reps=24, nmeas=12, lo_reps=1):
    """Per-exec device time via body repetition inside the NEFF: interleaved
    measurements of T(lo_reps) and T(reps); slope from median of differences."""
    import time
    import jax
    from jax.sharding import Mesh, PartitionSpec, NamedSharding

    devices = jax.devices()[:N_CORES]
    mesh = Mesh(np.asarray(devices), ("core",))
    shard = NamedSharding(mesh, PartitionSpec("core"))
    in_maps = make_in_maps(**inputs)

    def prep(nreps):
        r = _get_runner(nreps)
        base = _device_args(in_maps, nreps)
        n_params = r["n_params"]
        ins = [jax.device_put(a, shard) for a in base[:n_params]]
        zshapes = [a.shape for a in base[n_params:]]
        fn = r["make_multi"](1)

        def mz():
            return [jax.device_put(np.zeros(s, np.float32), shard)
                    for s in zshapes]
        jax.block_until_ready(fn(*ins, *mz()))  # warm / compile
        return fn, ins, mz

    fn_lo, ins_lo, mz_lo = prep(lo_reps)
    fn_hi, ins_hi, mz_hi = prep(reps)

    def timed(fn, ins, mz):
        zs = mz()
        jax.block_until_ready(zs)
        t0 = time.perf_counter()
        jax.block_until_ready(fn(*ins, *zs))
        return time.perf_counter() - t0

    diffs, los, his = [], [], []
    for _ in range(nmeas):
        tl = timed(fn_lo, ins_lo, mz_lo)
        th = timed(fn_hi, ins_hi, mz_hi)
        diffs.append(th - tl)
        los.append(tl)
        his.append(th)
    diffs.sort()
    med = diffs[len(diffs) // 2]
    per = med / (reps - lo_reps)
    return per, los, his


def bench(inputs, nreps=10, nloops=3):
    """Return estimated per-execution wall time in seconds (chained async
    dispatches; includes per-dispatch host/tunnel overhead)."""
    import time
    import jax
    from jax.sharding import Mesh, PartitionSpec, NamedSharding
    r = _get_runner()
    n_params = r["n_params"]
    in_maps = make_in_maps(**inputs)
    base = _device_args(in_maps)

    devices = jax.devices()[:N_CORES]
    mesh = Mesh(np.asarray(devices), ("core",))
    shard = NamedSharding(mesh, PartitionSpec("core"))

    ins = [jax.device_put(a, shard) for a in base[:n_params]]
    zero_shapes = [a.shape for a in base[n_params:]]

    def make_zeros():
        zs = [jax.device_put(np.zeros(s, np.float32), shard)
              for s in zero_shapes]
        for z in zs:
            z.block_until_ready()
        return zs

    run1 = r["run1"]
    y = run1(*ins, *make_zeros())  # warm up / compile
    jax.block_until_ready(y)

    def run_batch(n):
        zsets = [make_zeros() for _ in range(n)]
        jax.block_until_ready(ins)
        t0 = time.perf_counter()
        ys = [run1(*ins, *zs) for zs in zsets]
        jax.block_until_ready(ys)
        return time.perf_counter() - t0

    n_lo, n_hi = nreps, 3 * nreps
    best = float("inf")
    for _ in range(nloops):
        t_lo = run_batch(n_lo)
        t_hi = run_batch(n_hi)
        slope = (t_hi - t_lo) / (n_hi - n_lo)
        best = min(best, slope)
    return best


# revision 15
# speedup vs baseline: 6.3889x; 1.4753x over previous
"""Multi-head self-attention (b=2, n=2048, d_model=1024, 8 heads x 64) on 8 TRN2 cores.

Sharding: token-parallel (512 tokens/core, batch-major). K and V are exchanged
via three 4-rank AllGathers (replica groups = batch element), split so compute
can start as soon as each piece lands:

  AG-K  : KT shard  [512 inner, 512 tok]  -> scores can start after this alone
  AG-V0 : V_aug shard heads 0-3 [512 tok, 4*65]
  AG-V1 : V_aug shard heads 4-7 [512 tok, 4*65]

All layouts avoid transposing the attention matrix:

  xT    [1024, 512]  (PE-transposed from x shard)
  QT/KT [512(inner), tokens] = W.T @ xT   (matmul lhsT=W chunk, rhs=xT chunk)
  V_aug [tokens, 8*(64+1)]   = xT.T @ Wv  (+ ones column per head)
  scoresT[keys,q]  = matmul(lhsT=KT[64,128], rhs=QT[64,512])
  expT   = ACT exp(0.125*scoresT)  PSUM->SBUF
  outT[65,q]      += matmul(lhsT=V_aug[128,65], rhs=expT[128,512])  (row 64 = sumexp)
  normalize via DVE reciprocal + K=1 broadcast matmul
  y[tok,1024]      = matmul(lhsT=aoutT[128,128], rhs=Wo[128,512]) + ones x bo

AV matmuls are emitted LAG waves behind the score/exp stream so the (strictly
FIFO) PE queue never stalls waiting for the V gathers to land.
"""

import numpy as np

import concourse.bass as bass
import concourse.mybir as mybir
import concourse.tile as tile
from concourse import bacc
from concourse.bass_utils import run_bass_kernel_spmd
from concourse.masks import make_identity

F32 = mybir.dt.float32
FR = mybir.dt.float32r
BF = mybir.dt.bfloat16

B, S, D = 2, 2048, 1024
H, DH = 8, 64
INNER = H * DH            # 512
N_CORES = 8
GROUP = 4                 # cores per batch element
TOK = (B * S) // N_CORES  # 512 tokens per core
NKB = S // 128            # 16 key blocks per batch context
SCALE = DH ** -0.5        # 0.125
WAVE = 2                  # key blocks per score/exp wave
NW = NKB // WAVE          # 8 waves per head
LAG = 26                  # AV emission lag, in waves
VW = 4 * 65               # 260: V_aug columns per V gather half

REPLICA_GROUPS = [[0, 1, 2, 3], [4, 5, 6, 7]]

_CACHE = {}
NO_COLLECTIVE = False   # timing A/B switch (wrong math, same local work)


def _build_kernel(no_collective=False, reps=1):
    nc = bacc.Bacc("TRN2", target_bir_lowering=False, debug=False,
                   num_devices=N_CORES)

    x_d = nc.dram_tensor("x_shard", [TOK, D], F32, kind="ExternalInput")
    wq_d = nc.dram_tensor("Wq", [D, INNER], F32, kind="ExternalInput")
    wkv_d = nc.dram_tensor("Wkv", [D, 2 * INNER], F32, kind="ExternalInput")
    wo_d = nc.dram_tensor("Wo", [INNER, D], F32, kind="ExternalInput")
    bo_d = nc.dram_tensor("bo", [D], F32, kind="ExternalInput")
    y_d = nc.dram_tensor("y_shard", [TOK, D], F32, kind="ExternalOutput")

    agk0_in = nc.dram_tensor("agk0_in", [INNER // 2, TOK], BF,
                             kind="Internal")
    agk0_out = nc.dram_tensor("agk0_out", [GROUP * INNER // 2, TOK], BF,
                              kind="Internal")
    agk1_in = nc.dram_tensor("agk1_in", [INNER // 2, TOK], BF,
                             kind="Internal")
    agk1_out = nc.dram_tensor("agk1_out", [GROUP * INNER // 2, TOK], BF,
                              kind="Internal")
    agv0_in = nc.dram_tensor("agv0_in", [TOK, VW], BF, kind="Internal")
    agv0_out = nc.dram_tensor("agv0_out", [GROUP * TOK, VW], BF,
                              kind="Internal")
    agv1_in = nc.dram_tensor("agv1_in", [TOK, VW], BF, kind="Internal")
    agv1_out = nc.dram_tensor("agv1_out", [GROUP * TOK, VW], BF,
                              kind="Internal")

    with tile.TileContext(nc) as tc:
        for _ in range(reps):
            _trace_body(nc, tc, x_d, wq_d, wkv_d, wo_d, bo_d, y_d,
                        agk0_in, agk0_out, agk1_in, agk1_out,
                        agv0_in, agv0_out, agv1_in, agv1_out,
                        no_collective=no_collective)

    nc.compile()
    return nc


def _trace_body(nc, tc, x_d, wq_d, wkv_d, wo_d, bo_d, y_d,
                agk0_in, agk0_out, agk1_in, agk1_out,
                agv0_in, agv0_out, agv1_in, agv1_out,
                no_collective=False):
    Exp = mybir.ActivationFunctionType.Exp

    def fr(ap):
        # float32r: TF32-like PE mode, full-rate matmul at >=256 moving cols
        return ap.bitcast(mybir.dt.float32r)

    def all_gather(in_t, out_t, nrows):
        if no_collective:
            for r in range(GROUP):
                nc.sync.dma_start(
                    out_t.ap()[r * nrows:(r + 1) * nrows, :], in_t.ap())
        else:
            nc.gpsimd.collective_compute(
                "AllGather", mybir.AluOpType.bypass,
                replica_groups=REPLICA_GROUPS,
                ins=[in_t.ap()], outs=[out_t.ap()])

    with (
        tc.tile_pool(name="const", bufs=1) as constp,
        tc.tile_pool(name="wts", bufs=1) as wtsp,
        tc.tile_pool(name="qt", bufs=1) as qtp,
        tc.tile_pool(name="stage", bufs=3) as stagep,
        tc.tile_pool(name="expt", bufs=LAG + 3) as expp,
        tc.tile_pool(name="ao", bufs=1) as aop,
        tc.tile_pool(name="ys", bufs=2) as ysp,
        tc.tile_pool(name="small", bufs=2) as smallp,
    ):
        # ---- constants ----
        ident = constp.tile([128, 128], F32, tag="ident")
        make_identity(nc, ident[:])
        ones_f = constp.tile([1, 128], F32, tag="onesf")
        nc.gpsimd.memset(ones_f[:], 1.0)
        ones = constp.tile([1, 128], BF, tag="ones")
        nc.vector.tensor_copy(ones[:], ones_f[:])
        bo_sb = constp.tile([1, D], BF, tag="bo")

        # ---- persistent activations ----
        qt_sb = qtp.tile([128, 4, TOK], BF, tag="qt")          # QT [inner, tok]
        aout_sb = aop.tile([128, 4, TOK], BF, tag="aout")      # attnT out

        wo_sb = wtsp.tile([128, 4, D], BF, tag="wo")

        # ---- load x (two queues) and transpose: xT [1024, 512] ----
        wprojp_cm = tc.tile_pool(name="wproj", bufs=1)
        wprojp = wprojp_cm.__enter__()
        wkvk_sb = wprojp.tile([128, 8, INNER], BF, tag="wkvk")
        wkvv_sb = wprojp.tile([128, 8, INNER], BF, tag="wkvv")
        wq_sb = wprojp.tile([128, 8, INNER], BF, tag="wq")
        with (
            tc.tile_pool(name="xp", bufs=4) as xp,
            tc.tile_pool(name="xtp", bufs=1) as xtp,
            tc.tile_pool(name="pt", bufs=8, space="PSUM") as ptp,
        ):
            xt_sb = xtp.tile([128, 8, TOK], BF, tag="xt")
            x_tiles = []
            for a in range(4):
                x_t = xp.tile([128, D], F32, tag="x")
                eng = nc.sync if a < 2 else nc.scalar
                eng.dma_start(x_t[:], x_d.ap()[a * 128:(a + 1) * 128, :])
                x_tiles.append(x_t)

            # Weight loads (fp32->bf16 casting DMAs) must ride the
            # gpsimd queue; emitted BEFORE any collective so the triggers
            # fire before the gathers occupy the queue. Wkvk is split so the
            # m=0,1 half (heads 0-3) lands first and gates AG-K0 minimally.
            for half in range(2):
                nc.gpsimd.dma_start(
                    wkvk_sb[:, :, half * 256:(half + 1) * 256],
                    wkv_d.ap()[:, half * 256:half * 256 + 256]
                    .rearrange("(c p) n -> p c n", p=128))
            nc.gpsimd.dma_start(
                wkvv_sb[:],
                wkv_d.ap()[:, INNER:2 * INNER]
                .rearrange("(c p) n -> p c n", p=128))
            nc.gpsimd.dma_start(
                wq_sb[:], wq_d.ap().rearrange("(c p) n -> p c n", p=128))
            nc.gpsimd.dma_start(
                wo_sb[:], wo_d.ap().rearrange("(c p) n -> p c n", p=128))
            nc.gpsimd.dma_start(
                bo_sb[:], bo_d.ap().rearrange("(a n) -> a n", a=1))

            pts = [ptp.tile([128, TOK], F32, tag="pt", name=f"pt{c}")
                   for c in range(8)]
            for a in range(4):
                for c in range(8):
                    nc.tensor.transpose(
                        pts[c][:, a * 128:(a + 1) * 128],
                        x_tiles[a][:, c * 128:(c + 1) * 128], ident[:])
            for c in range(8):
                nc.vector.tensor_copy(xt_sb[:, c, :], pts[c][:])

        with tc.tile_pool(name="pworka", bufs=2, space="PSUM") as pworka:
            # ---- K projection -> agk0/agk1. AG-K0 (heads 0-3) is
            # issued as soon as its two m-chunks are staged so scores can
            # start ~a quarter-gather after the projections. ----
            for m in range(4):
                ps = pworka.tile([128, TOK], F32, tag="work")
                for c in range(8):
                    nc.tensor.matmul(ps[:],
                                     lhsT=wkvk_sb[:, c, m * 128:(m + 1) * 128],
                                     rhs=xt_sb[:, c, :],
                                     start=(c == 0), stop=(c == 7))
                st = stagep.tile([128, TOK], BF, tag="ktstage")
                nc.vector.tensor_copy(st[:], ps[:])
                agk_t = agk0_in if m < 2 else agk1_in
                nc.sync.dma_start(
                    agk_t.ap()[(m % 2) * 128:(m % 2) * 128 + 128, :], st[:])
                if m == 1:
                    all_gather(agk0_in, agk0_out, INNER // 2)

            # ---- V projection (+ones col) -> agv0/agv1, AllGather V ----
            for a in range(4):
                ps = pworka.tile([128, INNER], F32, tag="work")
                for c in range(8):
                    nc.tensor.matmul(ps[:],
                                     lhsT=xt_sb[:, c, a * 128:(a + 1) * 128],
                                     rhs=wkvv_sb[:, c, :],
                                     start=(c == 0), stop=(c == 7))
                vst = stagep.tile([128, H, 65], BF, tag="vstage")
                nc.vector.tensor_copy(
                    vst[:, :, 0:64], ps[:].rearrange("p (h e) -> p h e", e=64))
                nc.vector.memset(vst[:, :, 64:65], 1.0)
                nc.sync.dma_start(
                    agv0_in.ap()[a * 128:(a + 1) * 128, :]
                    .rearrange("p (h e) -> p h e", e=65),
                    vst[:, 0:4, :])
                nc.sync.dma_start(
                    agv1_in.ap()[a * 128:(a + 1) * 128, :]
                    .rearrange("p (h e) -> p h e", e=65),
                    vst[:, 4:8, :])
            all_gather(agv0_in, agv0_out, TOK)
            all_gather(agk1_in, agk1_out, INNER // 2)
            all_gather(agv1_in, agv1_out, TOK)

            # ---- Q projection (runs under the K gather) ----
            for m in range(4):
                ps = pworka.tile([128, TOK], F32, tag="work")
                for c in range(8):
                    nc.tensor.matmul(ps[:],
                                     lhsT=wq_sb[:, c, m * 128:(m + 1) * 128],
                                     rhs=xt_sb[:, c, :],
                                     start=(c == 0), stop=(c == 7))
                nc.vector.tensor_copy(qt_sb[:, m, :], ps[:])

        wprojp_cm.__exit__(None, None, None)

        # ---- load gathered K/V ----
        kvp_cm = tc.tile_pool(name="kv", bufs=1)
        kvp = kvp_cm.__enter__()
        kt_all = kvp.tile([128, 4, GROUP, TOK], BF, tag="kt")  # p, m, r, t
        for r in range(GROUP):
            nc.sync.dma_start(
                kt_all[:, 0:2, r, :],
                agk0_out.ap()[r * 256:(r + 1) * 256, :]
                .rearrange("(m p) t -> p m t", p=128))
        vaug_lo = kvp.tile([128, NKB, 4, 65], BF, tag="vlo")
        nc.sync.dma_start(
            vaug_lo[:],
            agv0_out.ap().rearrange("(kb p) (h e) -> p kb h e", p=128, e=65))
        for r in range(GROUP):
            nc.sync.dma_start(
                kt_all[:, 2:4, r, :],
                agk1_out.ap()[r * 256:(r + 1) * 256, :]
                .rearrange("(m p) t -> p m t", p=128))
        vaug_hi = kvp.tile([128, NKB, 4, 65], BF, tag="vhi")
        nc.sync.dma_start(
            vaug_hi[:],
            agv1_out.ap().rearrange("(kb p) (h e) -> p kb h e", p=128, e=65))

        # ---- attention, AV lagged by LAG waves ----
        pscorep_cm = tc.tile_pool(name="pscore", bufs=2, space="PSUM")
        pscorep = pscorep_cm.__enter__()
        pavp_cm = tc.tile_pool(name="pav", bufs=3, space="PSUM")
        pavp = pavp_cm.__enter__()
        pbp_cm = tc.tile_pool(name="pb", bufs=1, space="PSUM")
        pbp = pbp_cm.__enter__()

        def kt_slice(h, kb):
            po, m = (h % 2) * 64, h // 2
            return kt_all[po:po + 64, m, kb // 4,
                          (kb % 4) * 128:(kb % 4) * 128 + 128]

        def vaug_slice(h, kb):
            if h < 4:
                return vaug_lo[:, kb, h, :]
            return vaug_hi[:, kb, h - 4, :]

        def emit_scores(h, w):
            po, m = (h % 2) * 64, h // 2
            pscore = pscorep.tile([128, WAVE * TOK], F32, tag="s")
            for i in range(WAVE):
                kb = w * WAVE + i
                nc.tensor.matmul(
                    pscore[:, i * TOK:(i + 1) * TOK],
                    lhsT=kt_slice(h, kb),
                    rhs=qt_sb[po:po + 64, m, :],
                    start=True, stop=True)
            return pscore

        def emit_exp(pscore):
            expt = expp.tile([128, WAVE * TOK], BF, tag="expt")
            nc.scalar.activation(expt[:], pscore[:], Exp, scale=SCALE)
            return expt

        def emit_av(h, w, expt, pav):
            for i in range(WAVE):
                kb = w * WAVE + i
                nc.tensor.matmul(
                    pav[:],
                    lhsT=vaug_slice(h, kb),
                    rhs=expt[:, i * TOK:(i + 1) * TOK],
                    start=(kb == 0), stop=(kb == NKB - 1))

        def emit_normalize(h, pav):
            po, m = (h % 2) * 64, h // 2
            inv = smallp.tile([1, TOK], BF, tag="inv")
            with nc.allow_low_precision(reason="bf16 rounding of 1/sumexp"):
                nc.vector.reciprocal(inv[:], pav[64:65, :])
            pb = pbp.tile([64, TOK], F32, tag="pb")
            nc.tensor.matmul(pb[:], lhsT=ones[0:1, 0:64], rhs=inv[:],
                             start=True, stop=True)
            bcast = smallp.tile([64, TOK], F32, tag="bcast")
            nc.vector.tensor_copy(bcast[:], pb[:])
            nc.vector.tensor_mul(aout_sb[po:po + 64, m, :], pav[0:64, :],
                                 bcast[:])

        waves = [(h, w) for h in range(H) for w in range(NW)]
        pav_by_head = {}
        pending = []
        for g, (h, w) in enumerate(waves):
            pscore = emit_scores(h, w)
            expt = emit_exp(pscore)
            pending.append((h, w, expt))
            if g >= LAG:
                ph, pw, pexpt = pending[g - LAG]
                if ph not in pav_by_head:
                    pav_by_head[ph] = pavp.tile([65, TOK], F32, tag="av",
                                                name=f"pav{ph}")
                emit_av(ph, pw, pexpt, pav_by_head[ph])
                pending[g - LAG] = None
                if pw == NW - 1:
                    emit_normalize(ph, pav_by_head.pop(ph))
        for g in range(len(waves) - LAG, len(waves)):
            ph, pw, pexpt = pending[g]
            if ph not in pav_by_head:
                pav_by_head[ph] = pavp.tile([65, TOK], F32, tag="av",
                                            name=f"pav{ph}")
            emit_av(ph, pw, pexpt, pav_by_head[ph])
            if pw == NW - 1:
                emit_normalize(ph, pav_by_head.pop(ph))
        pbp_cm.__exit__(None, None, None)
        pavp_cm.__exit__(None, None, None)
        pscorep_cm.__exit__(None, None, None)
        kvp_cm.__exit__(None, None, None)

        # ---- output projection + bias ----
        with tc.tile_pool(name="pworkc", bufs=2, space="PSUM") as pworkc:
            for a in range(4):
                for j in range(2):
                    py = pworkc.tile([128, 512], F32, tag="work")
                    for c in range(4):
                        nc.tensor.matmul(
                            py[:],
                            lhsT=aout_sb[:, c, a * 128:(a + 1) * 128],
                            rhs=wo_sb[:, c, j * 512:(j + 1) * 512],
                            start=(c == 0), stop=False)
                    nc.tensor.matmul(py[:], lhsT=ones[0:1, :],
                                     rhs=bo_sb[0:1, j * 512:(j + 1) * 512],
                                     start=False, stop=True)
                    yst = ysp.tile([128, 512], F32, tag="ys")
                    nc.vector.tensor_copy(yst[:], py[:])
                    nc.sync.dma_start(
                        y_d.ap()[a * 128:(a + 1) * 128,
                                 j * 512:(j + 1) * 512],
                        yst[:])


def _get_nc(reps=1):
    key = ("nc", NO_COLLECTIVE, reps)
    if key not in _CACHE:
        _CACHE[key] = _build_kernel(no_collective=NO_COLLECTIVE, reps=reps)
    return _CACHE[key]


# ---------------------------------------------------------------------------
# Custom PJRT runner (mirrors bass2jax.run_bass_via_pjrt but builds the
# jitted executable once and keeps inputs device-resident so repeated calls
# measure device execution rather than host retrace/upload).
# ---------------------------------------------------------------------------

def _get_runner(reps=1):
    rkey = ("runner", NO_COLLECTIVE, reps)
    if rkey in _CACHE:
        return _CACHE[rkey]
    import jax
    from jax.sharding import Mesh, PartitionSpec
    from jax.experimental.shard_map import shard_map
    from concourse import bass2jax as b2j
    import concourse.mybir as mb

    nc = _get_nc(reps)
    b2j.install_neuronx_cc_hook()

    partition_name = (nc.partition_id_tensor.name
                      if nc.partition_id_tensor else None)

    in_names, out_names, out_avals, zero_outs = [], [], [], []
    for alloc in nc.m.functions[0].allocations:
        if not isinstance(alloc, mb.MemoryLocationSet):
            continue
        name = alloc.memorylocations[0].name
        if alloc.kind == "ExternalInput":
            if name != partition_name:
                in_names.append(name)
        elif alloc.kind == "ExternalOutput":
            shape = tuple(alloc.tensor_shape)
            dtype = mb.dt.np(alloc.dtype)
            out_names.append(name)
            out_avals.append(jax.core.ShapedArray(shape, dtype))
            zero_outs.append(np.zeros(shape, dtype))
    n_params = len(in_names)
    all_names = in_names + out_names
    if partition_name is not None:
        all_names = all_names + [partition_name]

    def _body(*args):
        operands = list(args)
        if partition_name is not None:
            operands.append(b2j.partition_id_tensor())
        outs = b2j._bass_exec_p.bind(
            *operands,
            out_avals=tuple(out_avals),
            in_names=tuple(all_names),
            out_names=tuple(out_names),
            lowering_input_output_aliases=(),
            sim_require_finite=True,
            sim_require_nnan=True,
            nc=nc,
        )
        return tuple(outs)

    devices = jax.devices()[:N_CORES]
    mesh = Mesh(np.asarray(devices), ("core",))
    nin = n_params + len(out_names)

    def _once(*args):
        return _body(*args)

    donate = tuple(range(n_params, nin))

    run1 = jax.jit(shard_map(
        _once, mesh=mesh,
        in_specs=(PartitionSpec("core"),) * nin,
        out_specs=(PartitionSpec("core"),) * len(out_names),
    ), donate_argnums=donate, keep_unused=True)

    n_outs = len(out_names)

    def _make_multi(ncalls):
        # N independent executions per dispatch; each call gets its own zero
        # output buffers (distinct params defeat XLA CSE), no donation.
        def _fn(*args):
            ins = args[:n_params]
            ys = []
            for i in range(ncalls):
                zeros = args[n_params + i * n_outs:
                             n_params + (i + 1) * n_outs]
                outs = _body(*ins, *zeros)
                ys.append(outs[0])
            return tuple(ys)
        return jax.jit(shard_map(
            _fn, mesh=mesh,
            in_specs=(PartitionSpec("core"),) * (n_params + ncalls * n_outs),
            out_specs=(PartitionSpec("core"),) * ncalls,
        ), keep_unused=True)

    runner = {
        "run1": run1, "make_multi": _make_multi,
        "in_names": in_names,
        "out_names": out_names, "zero_outs": zero_outs,
        "n_params": n_params,
    }
    _CACHE[rkey] = runner
    return runner


def _device_args(in_maps, reps=1):
    r = _get_runner(reps)
    concat = [np.concatenate([in_maps[c][n] for c in range(N_CORES)], axis=0)
              for n in r["in_names"]]
    zeros = [np.zeros((N_CORES * z.shape[0], *z.shape[1:]), z.dtype)
             for z in r["zero_outs"]]
    return concat + zeros


def make_in_maps(x, Wq, Wkv, Wo, bo):
    x_flat = np.ascontiguousarray(
        np.asarray(x, dtype=np.float32).reshape(B * S, D))
    Wq = np.ascontiguousarray(np.asarray(Wq, dtype=np.float32))
    Wkv = np.ascontiguousarray(np.asarray(Wkv, dtype=np.float32))
    Wo = np.ascontiguousarray(np.asarray(Wo, dtype=np.float32))
    bo = np.ascontiguousarray(np.asarray(bo, dtype=np.float32))
    return [
        {"x_shard": np.ascontiguousarray(x_flat[c * TOK:(c + 1) * TOK]),
         "Wq": Wq, "Wkv": Wkv, "Wo": Wo, "bo": bo}
        for c in range(N_CORES)
    ]


def kernel(x, Wq, Wkv, Wo, bo):
    r = _get_runner()
    in_maps = make_in_maps(x, Wq, Wkv, Wo, bo)
    args = _device_args(in_maps)
    outs = r["run1"](*args)
    y = np.asarray(outs[0])
    return y.reshape(B, S, D).astype(np.float32)


def bench3(inputs, reps=24, nmeas=12, lo_reps=1):
    """Per-exec device time via body repetition inside the NEFF: interleaved
    measurements of T(lo_reps) and T(reps); slope from median of differences."""
    import time
    import jax
    from jax.sharding import Mesh, PartitionSpec, NamedSharding

    devices = jax.devices()[:N_CORES]
    mesh = Mesh(np.asarray(devices), ("core",))
    shard = NamedSharding(mesh, PartitionSpec("core"))
    in_maps = make_in_maps(**inputs)

    def prep(nreps):
        r = _get_runner(nreps)
        base = _device_args(in_maps, nreps)
        n_params = r["n_params"]
        ins = [jax.device_put(a, shard) for a in base[:n_params]]
        zshapes = [a.shape for a in base[n_params:]]
        fn = r["make_multi"](1)

        def mz():
            return [jax.device_put(np.zeros(s, np.float32), shard)
                    for s in zshapes]
        jax.block_until_ready(fn(*ins, *mz()))  # warm / compile
        return fn, ins, mz

    fn_lo, ins_lo, mz_lo = prep(lo_reps)
    fn_hi, ins_hi, mz_hi = prep(reps)

    def timed(fn, ins, mz):
        zs = mz()
        jax.block_until_ready(zs)
        t0 = time.perf_counter()
        jax.block_until_ready(fn(*ins, *zs))
        return time.perf_counter() - t0

    diffs, los, his = [], [], []
    for _ in range(nmeas):
        tl = timed(fn_lo, ins_lo, mz_lo)
        th = timed(fn_hi, ins_hi, mz_hi)
        diffs.append(th - tl)
        los.append(tl)
        his.append(th)
    diffs.sort()
    med = diffs[len(diffs) // 2]
    per = med / (reps - lo_reps)
    return per, los, his


def bench(inputs, nreps=10, nloops=3):
    """Return estimated per-execution wall time in seconds (chained async
    dispatches; includes per-dispatch host/tunnel overhead)."""
    import time
    import jax
    from jax.sharding import Mesh, PartitionSpec, NamedSharding
    r = _get_runner()
    n_params = r["n_params"]
    in_maps = make_in_maps(**inputs)
    base = _device_args(in_maps)

    devices = jax.devices()[:N_CORES]
    mesh = Mesh(np.asarray(devices), ("core",))
    shard = NamedSharding(mesh, PartitionSpec("core"))

    ins = [jax.device_put(a, shard) for a in base[:n_params]]
    zero_shapes = [a.shape for a in base[n_params:]]

    def make_zeros():
        zs = [jax.device_put(np.zeros(s, np.float32), shard)
              for s in zero_shapes]
        for z in zs:
            z.block_until_ready()
        return zs

    run1 = r["run1"]
    y = run1(*ins, *make_zeros())  # warm up / compile
    jax.block_until_ready(y)

    def run_batch(n):
        zsets = [make_zeros() for _ in range(n)]
        jax.block_until_ready(ins)
        t0 = time.perf_counter()
        ys = [run1(*ins, *zs) for zs in zsets]
        jax.block_until_ready(ys)
        return time.perf_counter() - t0

    n_lo, n_hi = nreps, 3 * nreps
    best = float("inf")
    for _ in range(nloops):
        t_lo = run_batch(n_lo)
        t_hi = run_batch(n_hi)
        slope = (t_hi - t_lo) / (n_hi - n_lo)
        best = min(best, slope)
    return best
